# revision 14
# baseline (speedup 1.0000x reference)
"""Trainium2 Bass kernel for nn_EntityAlignmentModule.

Data-parallel over batch B=32 across 8 NeuronCores (4 samples/core).
All activations kept in transposed [feature, token] layout on chip;
LayerNorm statistics are computed with ones-vector matmuls on the
TensorEngine (partition-dim sums), gamma/beta are folded into the
following matmul's weights on the host, and the pairwise
relu(h_t[:,e] + h_i[:,r]) stage runs as broadcast-AP adds on the
Vector/GpSimd engines with the w2 contraction done on the TensorEngine
via per-batch one-hot weight columns accumulating into one [4, N] PSUM.
"""

import sys
import numpy as np

sys.path.insert(0, "/opt/trn_rl_repo")

import ml_dtypes  # noqa: E402
import concourse.bass as bass  # noqa: E402
import concourse.tile as tile  # noqa: E402
from concourse import bacc, mybir  # noqa: E402
from concourse.bass_utils import run_bass_kernel_spmd  # noqa: E402
from contextlib import ExitStack  # noqa: E402

AF = mybir.ActivationFunctionType
OP = mybir.AluOpType
F32 = mybir.dt.float32

B, E, R = 32, 64, 36
Dt, Di, D = 768, 2048, 512
LN_EPS = 1e-5
NCORES = 8
BL = B // NCORES          # 4 samples per core
TT = BL * E               # 256 text tokens per core
TI = BL * R               # 144 image tokens per core
KT = Dt // 128            # 6 text K chunks
KI = Di // 128            # 16 image K chunks
KD = D // 128             # 4 chunks of D
ER = E * R                # 2304 pairs per sample

# dtype mode for matmul operands: "bf16" (fast, ~5e-3 err) or
# "f32r" (TF32-like, ~2e-4 err, 2x DMA traffic)
DT_MODE = "bf16"
# pairwise tile ids (b*KD+c): adds on GpSimd for GP_TILES, else DVE;
# relus on DVE for DVE_RELU_TILES, else ACT
GP_TILES = tuple(int(x) for x in __import__("os").environ.get("KGP", "1,4,7,10,13").split(",") if x != "")
DVE_RELU_TILES = tuple(int(x) for x in __import__("os").environ.get("KDR", "0,1,2,3,4,5,6,7,8,9,10,11,12,13,14,15").split(",") if x != "")

_CACHE: dict = {}


def _register_relu_add():
    """Register a fused out = relu(in0 + in1) custom DVE op."""
    from concourse import dve_ops
    from concourse.dve_spec import Spec, Src0, Src1, relu, lower
    from concourse.dve_uop import DveOpSpec

    name = "RELU_ADD_KRN"
    if name in dve_ops._SUB_OPCODE_FOR_NAME:
        return next(op for op in dve_ops.OPS if op.name == name)
    spec = Spec(
        body=relu(Src0 + Src1),
        reference=lambda in0, in1, s0, s1, imm2: np.maximum(
            np.nan_to_num(in0.astype(np.float32) + in1), 0),
    )
    row = dve_ops._CUSTOM_DVE_ROW_BASE + len(dve_ops.OPS)
    assert row < 0x20
    shas = {}
    for ver in ("v3", "v4"):
        try:
            uops = lower(spec, ver=ver)
            shas[ver] = DveOpSpec(name=name, opcode=row, uops=uops,
                                  rd1_en=True).sha(ver)
        except Exception:
            pass
    op = dve_ops.DveOp(name, spec, subdim=False, uops_sha=shas)
    dve_ops.OPS.append(op)
    dve_ops.CUSTOM_DVE_SPECS[name] = spec
    dve_ops._SUB_OPCODE_FOR_NAME[name] = row
    return op


_RELU_ADD = _register_relu_add()


def _np_dt():
    return ml_dtypes.bfloat16 if DT_MODE == "bf16" else np.float32


def _bir_dt():
    return mybir.dt.bfloat16 if DT_MODE == "bf16" else mybir.dt.float32r


def _build_program():
    DT = _bir_dt()
    nc = bacc.Bacc()

    def par(name, shape, dt=None):
        return nc.declare_dram_parameter(name, list(shape), dt or DT, isOutput=False)

    # per-core activations, [128, chunks*tok] partition-major folded layouts
    xt = par("xt", [128, KT * TT])
    xi = par("xi", [128, KI * TI])
    # weights, [128, chunks*512]
    wt = par("wt", [128, KT * D])
    wi = par("wi", [128, KI * D])
    w1t = par("w1t", [128, KD * D])
    w1i = par("w1i", [128, KD * D])
    # one-hot w2 columns: [128, (c, b, 4)]
    w2oh = par("w2oh", [128, KD * BL * BL])
    # per-partition bias/scale columns (f32)
    btx = par("btx", [128, KD], F32)
    bim = par("bim", [128, KD], F32)
    gim = par("gim", [128, KD], F32)
    beE = par("beE", [128, KD], F32)
    b2c = par("b2c", [BL, 1], F32)
    biasrow_p = par("biasrow", [1, BL * D])
    indER_p = par("indER", [128, ER])
    epsc = par("epsc", [1, 1], F32)
    halfc = par("halfc", [BL, 1], F32)
    ones_c = par("ones_c", [128, 1])
    ones_r = par("ones_r", [1, 128])
    ident = par("ident", [128, 128])

    warm_out = nc.declare_dram_parameter("warm_out", [1, D], F32, isOutput=True)
    scores_out = nc.declare_dram_parameter("scores_out", [BL, E, R], F32, isOutput=True)
    agg_out = nc.declare_dram_parameter("agg_out", [BL, D], F32, isOutput=True)

    with ExitStack() as ctx:
        tc = tile.TileContext(nc)
        ctx.enter_context(tc)
        sb = ctx.enter_context(tc.tile_pool(name="sb", bufs=1))
        pp = tc.alloc_tile_pool(name="pp", bufs=2, space="PSUM")
        pstat = tc.alloc_tile_pool(name="pstat", bufs=1, space="PSUM")

        def load(name, p, shape, dt=None):
            t = sb.tile(list(shape), dt or DT, name=name)
            nc.sync.dma_start(t[:], p[:])
            return t

        xt_sb = load("xt_sb", xt, [128, KT * TT])
        xi_sb = load("xi_sb", xi, [128, KI * TI])
        wt_sb = load("wt_sb", wt, [128, KT * D])
        wi_sb = load("wi_sb", wi, [128, KI * D])
        w1t_sb = load("w1t_sb", w1t, [128, KD * D])
        w1i_sb = load("w1i_sb", w1i, [128, KD * D])
        w2_sb = load("w2_sb", w2oh, [128, KD * BL * BL])
        btx_sb = load("btx_sb", btx, [128, KD], F32)
        bim_sb = load("bim_sb", bim, [128, KD], F32)
        gim_sb = load("gim_sb", gim, [128, KD], F32)
        beE_sb = load("beE_sb", beE, [128, KD], F32)
        b2_sb = load("b2_sb", b2c, [BL, 1], F32)
        indER_sb = load("indER_sb", indER_p, [128, ER])
        eps_sb = load("eps_sb", epsc, [1, 1], F32)
        half_sb = load("half_sb", halfc, [BL, 1], F32)
        onesc_sb = load("onesc_sb", ones_c, [128, 1])
        onesr_sb = load("onesr_sb", ones_r, [1, 128])
        id_sb = load("id_sb", ident, [128, 128])

        # HAM warmup while input DMAs run: dense dummy matmuls on the ones col
        wps = pp.tile([1, D], F32, name="wps", tag="warm", bufs=1)
        for i in range(24):
            nc.tensor.matmul(wps[:], onesc_sb[:], w2_sb[:, 0:64].broadcast_to([128, D]) if False else wt_sb[0:128, 0:D],
                             start=(i == 0), stop=(i == 23))
        warm_sb = sb.tile([1, D], F32)
        nc.scalar.copy(warm_sb[:], wps[:])
        nc.sync.dma_start(warm_out[:], warm_sb[:])

        def proj_ln(x_sb, w_sb, bias_col, kch, tok, tag):
            """x (transposed, chunked) @ W -> relu -> LN normalize (no affine).
            Returns zn [128, KD*tok] in DT."""
            z = sb.tile([128, KD * tok], DT, name=f"z_{tag}")
            for m in range(KD):
                ps = pp.tile([128, tok], F32, name=f"ps_{tag}", tag="mm")
                for k in range(kch):
                    nc.tensor.matmul(
                        ps[:],
                        w_sb[:, k * D + m * 128:k * D + (m + 1) * 128],
                        x_sb[:, k * tok:(k + 1) * tok],
                        start=(k == 0), stop=(k == kch - 1),
                    )
                nc.scalar.activation(
                    z[:, m * tok:(m + 1) * tok], ps[:], AF.Relu,
                    bias=bias_col[:, m:m + 1], scale=1.0,
                )
            zsq = sb.tile([128, KD * tok], DT, name=f"zsq_{tag}")
            nc.scalar.activation(zsq[:], z[:], AF.Square)
            s1 = pstat.tile([1, tok], F32, name=f"s1_{tag}", tag="s1")
            s2 = pstat.tile([1, tok], F32, name=f"s2_{tag}", tag="s2")
            for m in range(KD):
                nc.tensor.matmul(s1[:], onesc_sb[:], z[:, m * tok:(m + 1) * tok],
                                 start=(m == 0), stop=(m == KD - 1))
            for m in range(KD):
                nc.tensor.matmul(s2[:], onesc_sb[:], zsq[:, m * tok:(m + 1) * tok],
                                 start=(m == 0), stop=(m == KD - 1))
            mean = sb.tile([1, tok], F32, name=f"mean_{tag}")
            nc.vector.tensor_scalar_mul(mean[:], s1[:], 1.0 / D)
            msq = sb.tile([1, tok], F32, name=f"msq_{tag}")
            nc.scalar.activation(msq[:], mean[:], AF.Square)
            var = sb.tile([1, tok], F32, name=f"var_{tag}")
            nc.vector.scalar_tensor_tensor(var[:], s2[:], 1.0 / D, msq[:],
                                           op0=OP.mult, op1=OP.subtract)
            lv = sb.tile([1, tok], F32, name=f"lv_{tag}")
            nc.scalar.activation(lv[:], var[:], AF.Ln, bias=eps_sb[0:1, 0:1], scale=1.0)
            a_row = sb.tile([1, tok], DT, name=f"a_{tag}")
            nc.scalar.activation(a_row[:], lv[:], AF.Exp, bias=0.0, scale=-0.5)
            c_row = sb.tile([1, tok], DT, name=f"c_{tag}")
            nc.vector.scalar_tensor_tensor(c_row[:], mean[:], -1.0, a_row[:],
                                           op0=OP.mult, op1=OP.mult)
            psA = pstat.tile([128, tok], F32, name=f"psA_{tag}", tag="psA")
            psC = pstat.tile([128, tok], F32, name=f"psC_{tag}", tag="psC")
            nc.tensor.matmul(psA[:], onesr_sb[:], a_row[:], start=True, stop=True)
            nc.tensor.matmul(psC[:], onesr_sb[:], c_row[:], start=True, stop=True)
            zn = sb.tile([128, KD * tok], DT, name=f"zn_{tag}")
            for m in range(KD):
                tmp = sb.tile([128, tok], F32, name=f"tmp_{tag}", tag=f"tmp_{tag}", bufs=2)
                nc.vector.tensor_tensor(tmp[:], z[:, m * tok:(m + 1) * tok], psA[:],
                                        op=OP.mult)
                nc.vector.tensor_tensor(zn[:, m * tok:(m + 1) * tok], tmp[:], psC[:],
                                        op=OP.add)
            return zn

        zn_t = proj_ln(xt_sb, wt_sb, btx_sb, KT, TT, "t")
        zn_i = proj_ln(xi_sb, wi_sb, bim_sb, KI, TI, "i")

        # h_t / h_i in standard [token, dhat] layout, packed into one tile:
        # pack[:, b*D + dh]: rows 0-63 = h_t[b, e, dh], rows 64-99 = h_i[b, r, dh],
        # row 100 = bias_ht + bias_hi (added via the indicator's ones row),
        # rows 101+ zeroed (indicator rows there are zero anyway).
        pack = sb.tile([128, BL * D], DT)
        nc.vector.memset(pack[96:128, :], 0.0)
        nc.sync.dma_start(pack[100:101, :], biasrow_p[:])
        for mt in range(2):  # text token tiles (128 tokens = 2 samples each)
            ps = pp.tile([128, D], F32, name="ps_ht", tag="mm")
            for k in range(KD):
                nc.tensor.matmul(ps[:], zn_t[:, k * TT + mt * 128:k * TT + (mt + 1) * 128],
                                 w1t_sb[:, k * D:(k + 1) * D],
                                 start=(k == 0), stop=(k == KD - 1))
            for j in range(2):
                b = 2 * mt + j
                nc.vector.tensor_copy(pack[0:64, b * D:(b + 1) * D],
                                      ps[j * 64:(j + 1) * 64, :])
        for b in range(BL):
            ps = pp.tile([36, D], F32, name="ps_hi", tag="mmi", bufs=1)
            for k in range(KD):
                nc.tensor.matmul(ps[:], zn_i[:, k * TI + b * R:k * TI + (b + 1) * R],
                                 w1i_sb[:, k * D:(k + 1) * D],
                                 start=(k == 0), stop=(k == KD - 1))
            nc.vector.tensor_copy(pack[64:100, b * D:(b + 1) * D], ps[:])

        pstat.release()
        pp.release()
        psc = ctx.enter_context(tc.tile_pool(name="psc", bufs=1, space="PSUM"))
        php = ctx.enter_context(tc.tile_pool(name="php", bufs=2, space="PSUM"))
        # pairwise: H = h_t[e] + h_i[r] + bias via one indicator matmul per
        # (b, c, ntile); relu-evac from PSUM; then w2 one-hot dot into sc_ps.
        NT = [(0, 512), (512, 512), (1024, 512), (1536, 512), (2048, 256)]
        sc_ps = [psc.tile([BL, n], F32, name=f"sc{i}", tag=f"sc{i}")
                 for i, (o, n) in enumerate(NT)]
        hp_pool = ctx.enter_context(tc.tile_pool(name="hp", bufs=3))
        first = True
        for b in range(BL):
            for c in range(KD):
                tid = b * KD + c
                hp2 = hp_pool.tile([128, ER], DT, name="hp2", tag="hp2")
                lhsH = pack[:, b * D + c * 128:b * D + (c + 1) * 128]
                lhsw = w2_sb[:, (c * BL + b) * BL:(c * BL + b + 1) * BL]
                for i, (o, n) in enumerate(NT):
                    hps = php.tile([128, n], F32, name="hps", tag="H")
                    nc.tensor.matmul(hps[:], lhsH, indER_sb[:, o:o + n],
                                     start=True, stop=True)
                    if tid in DVE_RELU_TILES:
                        nc.vector.tensor_scalar_max(hp2[:, o:o + n], hps[:], 0.0)
                    else:
                        nc.scalar.activation(hp2[:, o:o + n], hps[:], AF.Relu)
                    nc.tensor.matmul(sc_ps[i][:], lhsw, hp2[:, o:o + n],
                                     start=first, stop=(tid == BL * KD - 1))
                first = False

        tanh_sb = sb.tile([BL, ER], F32)
        for i, (o, n) in enumerate(NT):
            nc.scalar.activation(tanh_sb[:, o:o + n], sc_ps[i][:], AF.Tanh,
                                 bias=b2_sb[:, 0:1], scale=0.5)
        scores_sb = sb.tile([BL, ER], F32)
        nc.vector.tensor_scalar(scores_sb[:], tanh_sb[:], 0.5, op0=OP.mult,
                                scalar2=0.5, op1=OP.add)
        nc.sync.dma_start(scores_out[:], scores_sb[:].rearrange("b (e r) -> b e r", e=E))

        # softmax weights folded into u; exp(0.5*tanh + 0.5)
        exp_sb = sb.tile([BL, ER], DT)
        nc.scalar.activation(exp_sb[:], tanh_sb[:], AF.Exp,
                             bias=half_sb[:, 0:1], scale=0.5)
        u = sb.tile([BL, R], F32)
        nc.vector.tensor_reduce(u[:], exp_sb[:].rearrange("b (e r) -> b r e", e=E),
                                axis=mybir.AxisListType.X, op=OP.add)
        den = sb.tile([BL, 1], F32)
        nc.vector.tensor_reduce(den[:], u[:], axis=mybir.AxisListType.X, op=OP.add)
        rden = sb.tile([BL, 1], F32)
        nc.vector.reciprocal(rden[:], den[:])
        u_f = sb.tile([BL, R], DT)
        nc.vector.tensor_scalar(u_f[:], u[:], rden[:, 0:1], op0=OP.mult,
                                scalar2=1.0 / E, op1=OP.mult)
        u_row = sb.tile([1, TI], DT)
        nc.sync.dma_start(u_row[0:1, :].rearrange("q (b r) -> q b r", b=BL), u_f[:])

        psU = php.tile([128, TI], F32, tag="H")
        nc.tensor.matmul(psU[:], onesr_sb[:], u_row[:], start=True, stop=True)
        aggT = sb.tile([128, KD * BL], F32)
        for c in range(KD):
            tmp = sb.tile([128, TI], F32, name="agg_tmp", tag="agg_tmp", bufs=2)
            nc.vector.tensor_tensor(tmp[:], zn_i[:, c * TI:(c + 1) * TI], psU[:],
                                    op=OP.mult)
            nc.vector.tensor_reduce(
                aggT[:, c * BL:(c + 1) * BL],
                tmp[:].rearrange("p (b r) -> p b r", b=BL),
                axis=mybir.AxisListType.X, op=OP.add)
        aggF = sb.tile([128, KD * BL], DT)
        for c in range(KD):
            nc.vector.scalar_tensor_tensor(
                aggF[:, c * BL:(c + 1) * BL], aggT[:, c * BL:(c + 1) * BL],
                gim_sb[:, c:c + 1],
                beE_sb[:, c:c + 1].broadcast_to([128, BL]),
                op0=OP.mult, op1=OP.add)
        psT = php.tile([KD * BL, 128], DT, tag="H")
        nc.tensor.transpose(psT[:], aggF[:], id_sb[:])
        agg_sb = sb.tile([KD * BL, 128], F32)
        nc.scalar.copy(agg_sb[:], psT[:])
        # row (c*BL + b) -> agg_out[b, c*128 : (c+1)*128]
        for c in range(KD):
            nc.sync.dma_start(agg_out[:, c * 128:(c + 1) * 128],
                              agg_sb[c * BL:(c + 1) * BL, :])

    nc.compile()
    return nc


def _fold(a, nchunk):
    """[nchunk*128, cols] -> [128, nchunk*cols] partition-major layout."""
    n, cols = a.shape
    assert n == nchunk * 128
    return np.ascontiguousarray(
        a.reshape(nchunk, 128, cols).transpose(1, 0, 2).reshape(128, nchunk * cols))


def _prep_host(inputs):
    npdt = _np_dt()
    f32 = np.float32

    text = np.asarray(inputs["text_feats"], f32)
    image = np.asarray(inputs["image_feats"], f32)
    W_text = np.asarray(inputs["W_text"], f32)
    b_text = np.asarray(inputs["b_text"], f32)
    g_text = np.asarray(inputs["g_text"], f32)
    beta_text = np.asarray(inputs["beta_text"], f32)
    W_img = np.asarray(inputs["W_img"], f32)
    b_img = np.asarray(inputs["b_img"], f32)
    g_img = np.asarray(inputs["g_img"], f32)
    beta_img = np.asarray(inputs["beta_img"], f32)
    W1 = np.asarray(inputs["W1"], f32)
    b1 = np.asarray(inputs["b1"], f32)
    W2 = np.asarray(inputs["W2"], f32)
    b2 = np.asarray(inputs["b2"], f32)

    W1t = W1[:D] * g_text[:, None]
    W1i = W1[D:] * g_img[:, None]
    bht = beta_text @ W1[:D]
    bhi = beta_img @ W1[D:] + b1

    shared = {
        "wt": _fold(W_text, KT).astype(npdt),
        "wi": _fold(W_img, KI).astype(npdt),
        "w1t": _fold(W1t, KD).astype(npdt),
        "w1i": _fold(W1i, KD).astype(npdt),
        "btx": np.ascontiguousarray(b_text.reshape(KD, 128).T),
        "bim": np.ascontiguousarray(b_img.reshape(KD, 128).T),
        "biasrow": np.tile((bht + bhi), BL).reshape(1, BL * D).astype(npdt),
        "gim": np.ascontiguousarray(g_img.reshape(KD, 128).T),
        "beE": np.ascontiguousarray((beta_img / E).reshape(KD, 128).T),
        "b2c": np.full((BL, 1), b2[0] * 0.5, f32),
        "halfc": np.full((BL, 1), 0.5, f32),
        "epsc": np.full((1, 1), LN_EPS, f32),
        "ones_c": np.ones((128, 1), f32).astype(npdt),
        "ones_r": np.ones((1, 128), f32).astype(npdt),
        "ident": np.eye(128, dtype=f32).astype(npdt),
    }
    # one-hot w2: [128, (c, b, col)] where col b holds w2 chunk c
    w2f = W2[:, 0].reshape(KD, 128)
    w2oh = np.zeros((128, KD, BL, BL), f32)
    for c in range(KD):
        for b in range(BL):
            w2oh[:, c, b, b] = w2f[c]
    shared["w2oh"] = w2oh.reshape(128, KD * BL * BL).astype(npdt)
    ind = np.zeros((128, E, R), f32)
    for e in range(E):
        ind[e, e, :] = 1.0
    for r in range(R):
        ind[64 + r, :, r] = 1.0
    ind[100, :, :] = 1.0
    shared["indER"] = ind.reshape(128, ER).astype(npdt)

    in_maps = []
    for core in range(NCORES):
        sl = slice(core * BL, (core + 1) * BL)
        xt = _fold(np.ascontiguousarray(text[sl].reshape(TT, Dt).T), KT)
        xi = _fold(np.ascontiguousarray(image[sl].reshape(TI, Di).T), KI)
        m = dict(shared)
        m["xt"] = xt.astype(npdt)
        m["xi"] = xi.astype(npdt)
        in_maps.append(m)
    return in_maps


def _install_ntff_hook():
    """The slim container lacks antenv.axon_hooks; recreate it so
    run_bass_kernel_spmd(trace=True) can capture NTFF profiles."""
    import types, ctypes, contextlib

    try:
        from antenv.axon_hooks import get_axon_ntff_profile_hook  # noqa: F401
        return
    except ImportError:
        pass
    so_path = "/opt/axon/libaxon_pjrt.so"
    try:
        lib = ctypes.CDLL(so_path)
    except OSError:
        return
    if not hasattr(lib, "axon_start_nrt_profile"):
        return
    lib.axon_start_nrt_profile.argtypes = [ctypes.POINTER(ctypes.c_int64), ctypes.c_size_t]
    lib.axon_start_nrt_profile.restype = ctypes.c_int64
    lib.axon_stop_nrt_profile.argtypes = [ctypes.c_char_p]
    lib.axon_stop_nrt_profile.restype = ctypes.c_int64

    @contextlib.contextmanager
    def _hook(output_dir, device_ids):
        import jax
        jax.devices()
        if device_ids:
            ids = (ctypes.c_int64 * len(device_ids))(*device_ids)
            rc = lib.axon_start_nrt_profile(ids, len(device_ids))
        else:
            rc = lib.axon_start_nrt_profile(None, 0)
        if rc != 0:
            raise RuntimeError(f"axon_start_nrt_profile rc={rc}")
        try:
            yield
        finally:
            n = lib.axon_stop_nrt_profile(str(output_dir).encode())
            print(f"ntff profile: {n} file(s) -> {output_dir}", file=sys.stderr)

    mod = types.ModuleType("antenv.axon_hooks")
    mod.get_axon_ntff_profile_hook = lambda: _hook
    mod.set_axon_ntff_profile_hook = lambda h: None
    sys.modules["antenv.axon_hooks"] = mod
    import antenv
    antenv.axon_hooks = mod


def _get_program():
    key = DT_MODE
    if key not in _CACHE:
        _CACHE[key] = _build_program()
    return _CACHE[key]


def kernel(trace=False, **inputs):
    if trace:
        _install_ntff_hook()
    nc = _get_program()
    in_maps = _prep_host(inputs)
    res = run_bass_kernel_spmd(nc, in_maps, list(range(NCORES)), trace=trace)
    scores = np.concatenate([res.results[c]["scores_out"] for c in range(NCORES)], axis=0)
    agg = np.concatenate([res.results[c]["agg_out"] for c in range(NCORES)], axis=0)
    out = (scores.astype(np.float32), agg.astype(np.float32))
    if trace:
        return out, res
    return out


# revision 16
# speedup vs baseline: 1.0702x; 1.0702x over previous
"""Trainium2 Bass kernel for nn_EntityAlignmentModule.

Data-parallel over batch B=32 across 8 NeuronCores (4 samples/core).
All activations kept in transposed [feature, token] layout on chip;
LayerNorm statistics are computed with ones-vector matmuls on the
TensorEngine (partition-dim sums), gamma/beta are folded into the
following matmul's weights on the host, and the pairwise
relu(h_t[:,e] + h_i[:,r]) stage runs as broadcast-AP adds on the
Vector/GpSimd engines with the w2 contraction done on the TensorEngine
via per-batch one-hot weight columns accumulating into one [4, N] PSUM.
"""

import sys
import numpy as np

sys.path.insert(0, "/opt/trn_rl_repo")

import ml_dtypes  # noqa: E402
import concourse.bass as bass  # noqa: E402
import concourse.tile as tile  # noqa: E402
from concourse import bacc, mybir  # noqa: E402
from concourse.bass_utils import run_bass_kernel_spmd  # noqa: E402
from contextlib import ExitStack  # noqa: E402

AF = mybir.ActivationFunctionType
OP = mybir.AluOpType
F32 = mybir.dt.float32

B, E, R = 32, 64, 36
Dt, Di, D = 768, 2048, 512
LN_EPS = 1e-5
NCORES = 8
BL = B // NCORES          # 4 samples per core
TT = BL * E               # 256 text tokens per core
TI = BL * R               # 144 image tokens per core
KT = Dt // 128            # 6 text K chunks
KI = Di // 128            # 16 image K chunks
KD = D // 128             # 4 chunks of D
ER = E * R                # 2304 pairs per sample

# dtype mode for matmul operands: "bf16" (fast, ~5e-3 err) or
# "f32r" (TF32-like, ~2e-4 err, 2x DMA traffic)
DT_MODE = "bf16"
# pairwise tile ids (b*KD+c): adds on GpSimd for GP_TILES, else DVE;
# relus on DVE for DVE_RELU_TILES, else ACT
GP_TILES = tuple(int(x) for x in __import__("os").environ.get("KGP", "1,4,7,10,13").split(",") if x != "")
DVE_RELU_TILES = tuple(int(x) for x in __import__("os").environ.get("KDR", "0,2,4,6,8,10,12,14").split(",") if x != "")

_CACHE: dict = {}


def _register_relu_add():
    """Register a fused out = relu(in0 + in1) custom DVE op."""
    from concourse import dve_ops
    from concourse.dve_spec import Spec, Src0, Src1, relu, lower
    from concourse.dve_uop import DveOpSpec

    name = "RELU_ADD_KRN"
    if name in dve_ops._SUB_OPCODE_FOR_NAME:
        return next(op for op in dve_ops.OPS if op.name == name)
    spec = Spec(
        body=relu(Src0 + Src1),
        reference=lambda in0, in1, s0, s1, imm2: np.maximum(
            np.nan_to_num(in0.astype(np.float32) + in1), 0),
    )
    row = dve_ops._CUSTOM_DVE_ROW_BASE + len(dve_ops.OPS)
    assert row < 0x20
    shas = {}
    for ver in ("v3", "v4"):
        try:
            uops = lower(spec, ver=ver)
            shas[ver] = DveOpSpec(name=name, opcode=row, uops=uops,
                                  rd1_en=True).sha(ver)
        except Exception:
            pass
    op = dve_ops.DveOp(name, spec, subdim=False, uops_sha=shas)
    dve_ops.OPS.append(op)
    dve_ops.CUSTOM_DVE_SPECS[name] = spec
    dve_ops._SUB_OPCODE_FOR_NAME[name] = row
    return op


_RELU_ADD = _register_relu_add()


def _emit_slice(nc, item, sc_ps, w2_sb, si):
    tid, b, c, hp2, i, o, n, hps = item
    last = (tid == BL * KD - 1)
    if tid in DVE_RELU_TILES:
        nc.vector.tensor_scalar_max(hp2[:, o:o + n], hps[:], 0.0)
    else:
        nc.scalar.activation(hp2[:, o:o + n], hps[:], AF.Relu)
    nc.tensor.matmul(sc_ps[i][:], w2_sb[:, (c * BL + b) * BL:(c * BL + b + 1) * BL],
                     hp2[:, o:o + n], start=(tid == 0), stop=last)


def _np_dt():
    return ml_dtypes.bfloat16 if DT_MODE == "bf16" else np.float32


def _bir_dt():
    return mybir.dt.bfloat16 if DT_MODE == "bf16" else mybir.dt.float32r


def _build_program():
    DT = _bir_dt()
    nc = bacc.Bacc()

    def par(name, shape, dt=None):
        return nc.declare_dram_parameter(name, list(shape), dt or DT, isOutput=False)

    # per-core activations, [128, chunks*tok] partition-major folded layouts
    xt = par("xt", [128, KT * TT])
    xi = par("xi", [128, KI * TI])
    # weights, [128, chunks*512]
    wt = par("wt", [128, KT * D])
    wi = par("wi", [128, KI * D])
    w1t = par("w1t", [128, KD * D])
    w1i = par("w1i", [128, KD * D])
    # one-hot w2 columns: [128, (c, b, 4)]
    w2oh = par("w2oh", [128, KD * BL * BL])
    # per-partition bias/scale columns (f32)
    btx = par("btx", [128, KD], F32)
    bim = par("bim", [128, KD], F32)
    gim = par("gim", [128, KD], F32)
    beE = par("beE", [128, KD], F32)
    b2c = par("b2c", [BL, 1], F32)
    biasrow_p = par("biasrow", [1, BL * D])
    indER_p = par("indER", [128, ER])
    epsc = par("epsc", [1, 1], F32)
    halfc = par("halfc", [BL, 1], F32)
    ones_c = par("ones_c", [128, 1])
    ones_r = par("ones_r", [1, 128])
    ident = par("ident", [128, 128])

    warm_out = nc.declare_dram_parameter("warm_out", [1, D], F32, isOutput=True)
    scores_out = nc.declare_dram_parameter("scores_out", [BL, E, R], F32, isOutput=True)
    agg_out = nc.declare_dram_parameter("agg_out", [BL, D], F32, isOutput=True)

    with ExitStack() as ctx:
        tc = tile.TileContext(nc)
        ctx.enter_context(tc)
        sb = ctx.enter_context(tc.tile_pool(name="sb", bufs=1))
        pp = tc.alloc_tile_pool(name="pp", bufs=2, space="PSUM")
        pstat = tc.alloc_tile_pool(name="pstat", bufs=1, space="PSUM")

        def load(name, p, shape, dt=None):
            t = sb.tile(list(shape), dt or DT, name=name)
            nc.sync.dma_start(t[:], p[:])
            return t

        xt_sb = load("xt_sb", xt, [128, KT * TT])
        xi_sb = load("xi_sb", xi, [128, KI * TI])
        wt_sb = load("wt_sb", wt, [128, KT * D])
        wi_sb = load("wi_sb", wi, [128, KI * D])
        w1t_sb = load("w1t_sb", w1t, [128, KD * D])
        w1i_sb = load("w1i_sb", w1i, [128, KD * D])
        w2_sb = load("w2_sb", w2oh, [128, KD * BL * BL])
        btx_sb = load("btx_sb", btx, [128, KD], F32)
        bim_sb = load("bim_sb", bim, [128, KD], F32)
        gim_sb = load("gim_sb", gim, [128, KD], F32)
        beE_sb = load("beE_sb", beE, [128, KD], F32)
        b2_sb = load("b2_sb", b2c, [BL, 1], F32)
        indER_sb = load("indER_sb", indER_p, [128, ER])
        eps_sb = load("eps_sb", epsc, [1, 1], F32)
        half_sb = load("half_sb", halfc, [BL, 1], F32)
        onesc_sb = load("onesc_sb", ones_c, [128, 1])
        onesr_sb = load("onesr_sb", ones_r, [1, 128])
        id_sb = load("id_sb", ident, [128, 128])

        # HAM warmup while input DMAs run: dense dummy matmuls on the ones col
        wps = pp.tile([1, D], F32, name="wps", tag="warm", bufs=1)
        for i in range(24):
            nc.tensor.matmul(wps[:], onesc_sb[:], w2_sb[:, 0:64].broadcast_to([128, D]) if False else wt_sb[0:128, 0:D],
                             start=(i == 0), stop=(i == 23))
        warm_sb = sb.tile([1, D], F32)
        nc.scalar.copy(warm_sb[:], wps[:])
        nc.sync.dma_start(warm_out[:], warm_sb[:])

        def proj_ln(x_sb, w_sb, bias_col, kch, tok, tag):
            """x (transposed, chunked) @ W -> relu -> LN normalize (no affine).
            Returns zn [128, KD*tok] in DT."""
            z = sb.tile([128, KD * tok], DT, name=f"z_{tag}")
            for m in range(KD):
                ps = pp.tile([128, tok], F32, name=f"ps_{tag}", tag="mm")
                for k in range(kch):
                    nc.tensor.matmul(
                        ps[:],
                        w_sb[:, k * D + m * 128:k * D + (m + 1) * 128],
                        x_sb[:, k * tok:(k + 1) * tok],
                        start=(k == 0), stop=(k == kch - 1),
                    )
                nc.scalar.activation(
                    z[:, m * tok:(m + 1) * tok], ps[:], AF.Relu,
                    bias=bias_col[:, m:m + 1], scale=1.0,
                )
            zsq = sb.tile([128, KD * tok], DT, name=f"zsq_{tag}")
            nc.scalar.activation(zsq[:], z[:], AF.Square)
            s1 = pstat.tile([1, tok], F32, name=f"s1_{tag}", tag="s1")
            s2 = pstat.tile([1, tok], F32, name=f"s2_{tag}", tag="s2")
            for m in range(KD):
                nc.tensor.matmul(s1[:], onesc_sb[:], z[:, m * tok:(m + 1) * tok],
                                 start=(m == 0), stop=(m == KD - 1))
            for m in range(KD):
                nc.tensor.matmul(s2[:], onesc_sb[:], zsq[:, m * tok:(m + 1) * tok],
                                 start=(m == 0), stop=(m == KD - 1))
            mean = sb.tile([1, tok], F32, name=f"mean_{tag}")
            nc.vector.tensor_scalar_mul(mean[:], s1[:], 1.0 / D)
            msq = sb.tile([1, tok], F32, name=f"msq_{tag}")
            nc.scalar.activation(msq[:], mean[:], AF.Square)
            var = sb.tile([1, tok], F32, name=f"var_{tag}")
            nc.vector.scalar_tensor_tensor(var[:], s2[:], 1.0 / D, msq[:],
                                           op0=OP.mult, op1=OP.subtract)
            lv = sb.tile([1, tok], F32, name=f"lv_{tag}")
            nc.scalar.activation(lv[:], var[:], AF.Ln, bias=eps_sb[0:1, 0:1], scale=1.0)
            a_row = sb.tile([1, tok], DT, name=f"a_{tag}")
            nc.scalar.activation(a_row[:], lv[:], AF.Exp, bias=0.0, scale=-0.5)
            c_row = sb.tile([1, tok], DT, name=f"c_{tag}")
            nc.vector.scalar_tensor_tensor(c_row[:], mean[:], -1.0, a_row[:],
                                           op0=OP.mult, op1=OP.mult)
            psA = pstat.tile([128, tok], F32, name=f"psA_{tag}", tag="psA")
            psC = pstat.tile([128, tok], F32, name=f"psC_{tag}", tag="psC")
            nc.tensor.matmul(psA[:], onesr_sb[:], a_row[:], start=True, stop=True)
            nc.tensor.matmul(psC[:], onesr_sb[:], c_row[:], start=True, stop=True)
            zn = sb.tile([128, KD * tok], DT, name=f"zn_{tag}")
            for m in range(KD):
                tmp = sb.tile([128, tok], F32, name=f"tmp_{tag}", tag=f"tmp_{tag}", bufs=2)
                nc.vector.tensor_tensor(tmp[:], z[:, m * tok:(m + 1) * tok], psA[:],
                                        op=OP.mult)
                nc.vector.tensor_tensor(zn[:, m * tok:(m + 1) * tok], tmp[:], psC[:],
                                        op=OP.add)
            return zn

        zn_t = proj_ln(xt_sb, wt_sb, btx_sb, KT, TT, "t")
        zn_i = proj_ln(xi_sb, wi_sb, bim_sb, KI, TI, "i")

        # h_t / h_i in standard [token, dhat] layout, packed into one tile:
        # pack[:, b*D + dh]: rows 0-63 = h_t[b, e, dh], rows 64-99 = h_i[b, r, dh],
        # row 100 = bias_ht + bias_hi (added via the indicator's ones row),
        # rows 101+ zeroed (indicator rows there are zero anyway).
        pack = sb.tile([128, BL * D], DT)
        nc.vector.memset(pack[96:128, :], 0.0)
        nc.sync.dma_start(pack[100:101, :], biasrow_p[:])
        for mt in range(2):  # text token tiles (128 tokens = 2 samples each)
            ps = pp.tile([128, D], F32, name="ps_ht", tag="mm")
            for k in range(KD):
                nc.tensor.matmul(ps[:], zn_t[:, k * TT + mt * 128:k * TT + (mt + 1) * 128],
                                 w1t_sb[:, k * D:(k + 1) * D],
                                 start=(k == 0), stop=(k == KD - 1))
            for j in range(2):
                b = 2 * mt + j
                nc.vector.tensor_copy(pack[0:64, b * D:(b + 1) * D],
                                      ps[j * 64:(j + 1) * 64, :])
        for b in range(BL):
            ps = pp.tile([36, D], F32, name="ps_hi", tag="mmi", bufs=1)
            for k in range(KD):
                nc.tensor.matmul(ps[:], zn_i[:, k * TI + b * R:k * TI + (b + 1) * R],
                                 w1i_sb[:, k * D:(k + 1) * D],
                                 start=(k == 0), stop=(k == KD - 1))
            nc.vector.tensor_copy(pack[64:100, b * D:(b + 1) * D], ps[:])

        pstat.release()
        pp.release()
        psc = ctx.enter_context(tc.tile_pool(name="psc", bufs=1, space="PSUM"))
        php = ctx.enter_context(tc.tile_pool(name="php", bufs=3, space="PSUM"))
        # pairwise: H = h_t[e] + h_i[r] + bias via one indicator matmul per
        # (b, c, ntile); relu-evac from PSUM; then w2 one-hot dot into sc_ps.
        NT = [(0, 512), (512, 512), (1024, 512), (1536, 512), (2048, 256)]
        sc_ps = [psc.tile([BL, n], F32, name=f"sc{i}", tag=f"sc{i}")
                 for i, (o, n) in enumerate(NT)]
        hp_pool = ctx.enter_context(tc.tile_pool(name="hp", bufs=3))
        # software-pipelined at H-slice granularity so the dot matmul for
        # slice k never blocks the expansion matmul for slice k+1 in PE's FIFO
        slices = []
        for b in range(BL):
            for c in range(KD):
                tid = b * KD + c
                hp2 = hp_pool.tile([128, ER], DT, name="hp2", tag="hp2")
                for i, (o, n) in enumerate(NT):
                    slices.append((tid, b, c, hp2, i, o, n))
        pend = []
        for si, (tid, b, c, hp2, i, o, n) in enumerate(slices):
            hps = php.tile([128, n], F32, name="hps", tag="H")
            nc.tensor.matmul(hps[:], pack[:, b * D + c * 128:b * D + (c + 1) * 128],
                             indER_sb[:, o:o + n], start=True, stop=True)
            pend.append((tid, b, c, hp2, i, o, n, hps))
            if len(pend) >= 2:
                _emit_slice(nc, pend.pop(0), sc_ps, w2_sb, si)
        while pend:
            _emit_slice(nc, pend.pop(0), sc_ps, w2_sb, -1)

        tanh_sb = sb.tile([BL, ER], F32)
        for i, (o, n) in enumerate(NT):
            nc.scalar.activation(tanh_sb[:, o:o + n], sc_ps[i][:], AF.Tanh,
                                 bias=b2_sb[:, 0:1], scale=0.5)
        scores_sb = sb.tile([BL, ER], F32)
        nc.vector.tensor_scalar(scores_sb[:], tanh_sb[:], 0.5, op0=OP.mult,
                                scalar2=0.5, op1=OP.add)
        nc.sync.dma_start(scores_out[:], scores_sb[:].rearrange("b (e r) -> b e r", e=E))

        # softmax weights folded into u; exp(0.5*tanh + 0.5)
        exp_sb = sb.tile([BL, ER], DT)
        nc.scalar.activation(exp_sb[:], tanh_sb[:], AF.Exp,
                             bias=half_sb[:, 0:1], scale=0.5)
        u = sb.tile([BL, R], F32)
        nc.vector.tensor_reduce(u[:], exp_sb[:].rearrange("b (e r) -> b r e", e=E),
                                axis=mybir.AxisListType.X, op=OP.add)
        den = sb.tile([BL, 1], F32)
        nc.vector.tensor_reduce(den[:], u[:], axis=mybir.AxisListType.X, op=OP.add)
        rden = sb.tile([BL, 1], F32)
        nc.vector.reciprocal(rden[:], den[:])
        u_f = sb.tile([BL, R], DT)
        nc.vector.tensor_scalar(u_f[:], u[:], rden[:, 0:1], op0=OP.mult,
                                scalar2=1.0 / E, op1=OP.mult)
        u_row = sb.tile([1, TI], DT)
        nc.sync.dma_start(u_row[0:1, :].rearrange("q (b r) -> q b r", b=BL), u_f[:])

        psU = php.tile([128, TI], F32, tag="H")
        nc.tensor.matmul(psU[:], onesr_sb[:], u_row[:], start=True, stop=True)
        aggT = sb.tile([128, KD * BL], F32)
        for c in range(KD):
            tmp = sb.tile([128, TI], F32, name="agg_tmp", tag="agg_tmp", bufs=2)
            nc.vector.tensor_tensor(tmp[:], zn_i[:, c * TI:(c + 1) * TI], psU[:],
                                    op=OP.mult)
            nc.vector.tensor_reduce(
                aggT[:, c * BL:(c + 1) * BL],
                tmp[:].rearrange("p (b r) -> p b r", b=BL),
                axis=mybir.AxisListType.X, op=OP.add)
        aggF = sb.tile([128, KD * BL], DT)
        for c in range(KD):
            nc.vector.scalar_tensor_tensor(
                aggF[:, c * BL:(c + 1) * BL], aggT[:, c * BL:(c + 1) * BL],
                gim_sb[:, c:c + 1],
                beE_sb[:, c:c + 1].broadcast_to([128, BL]),
                op0=OP.mult, op1=OP.add)
        psT = php.tile([KD * BL, 128], DT, tag="H")
        nc.tensor.transpose(psT[:], aggF[:], id_sb[:])
        agg_sb = sb.tile([KD * BL, 128], F32)
        nc.scalar.copy(agg_sb[:], psT[:])
        # row (c*BL + b) -> agg_out[b, c*128 : (c+1)*128]
        for c in range(KD):
            nc.sync.dma_start(agg_out[:, c * 128:(c + 1) * 128],
                              agg_sb[c * BL:(c + 1) * BL, :])

    nc.compile()
    return nc


def _fold(a, nchunk):
    """[nchunk*128, cols] -> [128, nchunk*cols] partition-major layout."""
    n, cols = a.shape
    assert n == nchunk * 128
    return np.ascontiguousarray(
        a.reshape(nchunk, 128, cols).transpose(1, 0, 2).reshape(128, nchunk * cols))


def _prep_host(inputs):
    npdt = _np_dt()
    f32 = np.float32

    text = np.asarray(inputs["text_feats"], f32)
    image = np.asarray(inputs["image_feats"], f32)
    W_text = np.asarray(inputs["W_text"], f32)
    b_text = np.asarray(inputs["b_text"], f32)
    g_text = np.asarray(inputs["g_text"], f32)
    beta_text = np.asarray(inputs["beta_text"], f32)
    W_img = np.asarray(inputs["W_img"], f32)
    b_img = np.asarray(inputs["b_img"], f32)
    g_img = np.asarray(inputs["g_img"], f32)
    beta_img = np.asarray(inputs["beta_img"], f32)
    W1 = np.asarray(inputs["W1"], f32)
    b1 = np.asarray(inputs["b1"], f32)
    W2 = np.asarray(inputs["W2"], f32)
    b2 = np.asarray(inputs["b2"], f32)

    W1t = W1[:D] * g_text[:, None]
    W1i = W1[D:] * g_img[:, None]
    bht = beta_text @ W1[:D]
    bhi = beta_img @ W1[D:] + b1

    shared = {
        "wt": _fold(W_text, KT).astype(npdt),
        "wi": _fold(W_img, KI).astype(npdt),
        "w1t": _fold(W1t, KD).astype(npdt),
        "w1i": _fold(W1i, KD).astype(npdt),
        "btx": np.ascontiguousarray(b_text.reshape(KD, 128).T),
        "bim": np.ascontiguousarray(b_img.reshape(KD, 128).T),
        "biasrow": np.tile((bht + bhi), BL).reshape(1, BL * D).astype(npdt),
        "gim": np.ascontiguousarray(g_img.reshape(KD, 128).T),
        "beE": np.ascontiguousarray((beta_img / E).reshape(KD, 128).T),
        "b2c": np.full((BL, 1), b2[0] * 0.5, f32),
        "halfc": np.full((BL, 1), 0.5, f32),
        "epsc": np.full((1, 1), LN_EPS, f32),
        "ones_c": np.ones((128, 1), f32).astype(npdt),
        "ones_r": np.ones((1, 128), f32).astype(npdt),
        "ident": np.eye(128, dtype=f32).astype(npdt),
    }
    # one-hot w2: [128, (c, b, col)] where col b holds w2 chunk c
    w2f = W2[:, 0].reshape(KD, 128)
    w2oh = np.zeros((128, KD, BL, BL), f32)
    for c in range(KD):
        for b in range(BL):
            w2oh[:, c, b, b] = w2f[c]
    shared["w2oh"] = w2oh.reshape(128, KD * BL * BL).astype(npdt)
    ind = np.zeros((128, E, R), f32)
    for e in range(E):
        ind[e, e, :] = 1.0
    for r in range(R):
        ind[64 + r, :, r] = 1.0
    ind[100, :, :] = 1.0
    shared["indER"] = ind.reshape(128, ER).astype(npdt)

    in_maps = []
    for core in range(NCORES):
        sl = slice(core * BL, (core + 1) * BL)
        xt = _fold(np.ascontiguousarray(text[sl].reshape(TT, Dt).T), KT)
        xi = _fold(np.ascontiguousarray(image[sl].reshape(TI, Di).T), KI)
        m = dict(shared)
        m["xt"] = xt.astype(npdt)
        m["xi"] = xi.astype(npdt)
        in_maps.append(m)
    return in_maps


def _install_ntff_hook():
    """The slim container lacks antenv.axon_hooks; recreate it so
    run_bass_kernel_spmd(trace=True) can capture NTFF profiles."""
    import types, ctypes, contextlib

    try:
        from antenv.axon_hooks import get_axon_ntff_profile_hook  # noqa: F401
        return
    except ImportError:
        pass
    so_path = "/opt/axon/libaxon_pjrt.so"
    try:
        lib = ctypes.CDLL(so_path)
    except OSError:
        return
    if not hasattr(lib, "axon_start_nrt_profile"):
        return
    lib.axon_start_nrt_profile.argtypes = [ctypes.POINTER(ctypes.c_int64), ctypes.c_size_t]
    lib.axon_start_nrt_profile.restype = ctypes.c_int64
    lib.axon_stop_nrt_profile.argtypes = [ctypes.c_char_p]
    lib.axon_stop_nrt_profile.restype = ctypes.c_int64

    @contextlib.contextmanager
    def _hook(output_dir, device_ids):
        import jax
        jax.devices()
        if device_ids:
            ids = (ctypes.c_int64 * len(device_ids))(*device_ids)
            rc = lib.axon_start_nrt_profile(ids, len(device_ids))
        else:
            rc = lib.axon_start_nrt_profile(None, 0)
        if rc != 0:
            raise RuntimeError(f"axon_start_nrt_profile rc={rc}")
        try:
            yield
        finally:
            n = lib.axon_stop_nrt_profile(str(output_dir).encode())
            print(f"ntff profile: {n} file(s) -> {output_dir}", file=sys.stderr)

    mod = types.ModuleType("antenv.axon_hooks")
    mod.get_axon_ntff_profile_hook = lambda: _hook
    mod.set_axon_ntff_profile_hook = lambda h: None
    sys.modules["antenv.axon_hooks"] = mod
    import antenv
    antenv.axon_hooks = mod


def _get_program():
    key = DT_MODE
    if key not in _CACHE:
        _CACHE[key] = _build_program()
    return _CACHE[key]


def kernel(trace=False, **inputs):
    if trace:
        _install_ntff_hook()
    nc = _get_program()
    in_maps = _prep_host(inputs)
    res = run_bass_kernel_spmd(nc, in_maps, list(range(NCORES)), trace=trace)
    scores = np.concatenate([res.results[c]["scores_out"] for c in range(NCORES)], axis=0)
    agg = np.concatenate([res.results[c]["agg_out"] for c in range(NCORES)], axis=0)
    out = (scores.astype(np.float32), agg.astype(np.float32))
    if trace:
        return out, res
    return out


# revision 17
# speedup vs baseline: 1.1016x; 1.0293x over previous
"""Trainium2 Bass kernel for nn_EntityAlignmentModule.

Data-parallel over batch B=32 across 8 NeuronCores (4 samples/core).
All activations kept in transposed [feature, token] layout on chip;
LayerNorm statistics are computed with ones-vector matmuls on the
TensorEngine (partition-dim sums), gamma/beta are folded into the
following matmul's weights on the host, and the pairwise
relu(h_t[:,e] + h_i[:,r]) stage runs as broadcast-AP adds on the
Vector/GpSimd engines with the w2 contraction done on the TensorEngine
via per-batch one-hot weight columns accumulating into one [4, N] PSUM.
"""

import sys
import numpy as np

sys.path.insert(0, "/opt/trn_rl_repo")

import ml_dtypes  # noqa: E402
import concourse.bass as bass  # noqa: E402
import concourse.tile as tile  # noqa: E402
from concourse import bacc, mybir  # noqa: E402
from concourse.bass_utils import run_bass_kernel_spmd  # noqa: E402
from contextlib import ExitStack  # noqa: E402

AF = mybir.ActivationFunctionType
OP = mybir.AluOpType
F32 = mybir.dt.float32

B, E, R = 32, 64, 36
Dt, Di, D = 768, 2048, 512
LN_EPS = 1e-5
NCORES = 8
BL = B // NCORES          # 4 samples per core
TT = BL * E               # 256 text tokens per core
TI = BL * R               # 144 image tokens per core
KT = Dt // 128            # 6 text K chunks
KI = Di // 128            # 16 image K chunks
KD = D // 128             # 4 chunks of D
ER = E * R                # 2304 pairs per sample

# dtype mode for matmul operands: "bf16" (fast, ~5e-3 err) or
# "f32r" (TF32-like, ~2e-4 err, 2x DMA traffic)
DT_MODE = "bf16"
# pairwise tile ids (b*KD+c): adds on GpSimd for GP_TILES, else DVE;
# relus on DVE for DVE_RELU_TILES, else ACT
GP_TILES = tuple(int(x) for x in __import__("os").environ.get("KGP", "1,4,7,10,13").split(",") if x != "")
DVE_RELU_TILES = tuple(int(x) for x in __import__("os").environ.get("KDR", "0,2,4,6,8,10,12,14").split(",") if x != "")

_CACHE: dict = {}


def _register_relu_add():
    """Register a fused out = relu(in0 + in1) custom DVE op."""
    from concourse import dve_ops
    from concourse.dve_spec import Spec, Src0, Src1, relu, lower
    from concourse.dve_uop import DveOpSpec

    name = "RELU_ADD_KRN"
    if name in dve_ops._SUB_OPCODE_FOR_NAME:
        return next(op for op in dve_ops.OPS if op.name == name)
    spec = Spec(
        body=relu(Src0 + Src1),
        reference=lambda in0, in1, s0, s1, imm2: np.maximum(
            np.nan_to_num(in0.astype(np.float32) + in1), 0),
    )
    row = dve_ops._CUSTOM_DVE_ROW_BASE + len(dve_ops.OPS)
    assert row < 0x20
    shas = {}
    for ver in ("v3", "v4"):
        try:
            uops = lower(spec, ver=ver)
            shas[ver] = DveOpSpec(name=name, opcode=row, uops=uops,
                                  rd1_en=True).sha(ver)
        except Exception:
            pass
    op = dve_ops.DveOp(name, spec, subdim=False, uops_sha=shas)
    dve_ops.OPS.append(op)
    dve_ops.CUSTOM_DVE_SPECS[name] = spec
    dve_ops._SUB_OPCODE_FOR_NAME[name] = row
    return op


_RELU_ADD = _register_relu_add()


def _emit_slice(nc, item, sc_ps, w2_sb, si):
    tid, b, c, hp2, i, o, n, hps = item
    last = (tid == BL * KD - 1)
    if si % 2 == 0:
        nc.vector.tensor_scalar_max(hp2[:, o:o + n], hps[:], 0.0)
    else:
        nc.scalar.activation(hp2[:, o:o + n], hps[:], AF.Relu)
    nc.tensor.matmul(sc_ps[i][:], w2_sb[:, (c * BL + b) * BL:(c * BL + b + 1) * BL],
                     hp2[:, o:o + n], start=(tid == 0), stop=last)


def _np_dt():
    return ml_dtypes.bfloat16 if DT_MODE == "bf16" else np.float32


def _bir_dt():
    return mybir.dt.bfloat16 if DT_MODE == "bf16" else mybir.dt.float32r


def _build_program():
    DT = _bir_dt()
    nc = bacc.Bacc()

    def par(name, shape, dt=None):
        return nc.declare_dram_parameter(name, list(shape), dt or DT, isOutput=False)

    # per-core activations, [128, chunks*tok] partition-major folded layouts
    xt = par("xt", [128, KT * TT])
    xi = par("xi", [128, KI * TI])
    # weights, [128, chunks*512]
    wt = par("wt", [128, KT * D])
    wi = par("wi", [128, KI * D])
    w1t = par("w1t", [128, KD * D])
    w1i = par("w1i", [128, KD * D])
    # one-hot w2 columns: [128, (c, b, 4)]
    w2oh = par("w2oh", [128, KD * BL * BL])
    # per-partition bias/scale columns (f32)
    btx = par("btx", [128, KD], F32)
    bim = par("bim", [128, KD], F32)
    gim = par("gim", [128, KD], F32)
    beE = par("beE", [128, KD], F32)
    b2c = par("b2c", [BL, 1], F32)
    biasrow_p = par("biasrow", [1, BL * D])
    indER_p = par("indER", [128, ER])
    epsc = par("epsc", [1, 1], F32)
    halfc = par("halfc", [BL, 1], F32)
    ones_c = par("ones_c", [128, 1])
    ones_r = par("ones_r", [1, 128])
    ident = par("ident", [128, 128])

    warm_out = nc.declare_dram_parameter("warm_out", [1, D], F32, isOutput=True)
    scores_out = nc.declare_dram_parameter("scores_out", [BL, E, R], F32, isOutput=True)
    agg_out = nc.declare_dram_parameter("agg_out", [BL, D], F32, isOutput=True)

    with ExitStack() as ctx:
        tc = tile.TileContext(nc)
        ctx.enter_context(tc)
        sb = ctx.enter_context(tc.tile_pool(name="sb", bufs=1))
        pp = tc.alloc_tile_pool(name="pp", bufs=2, space="PSUM")
        pstat = tc.alloc_tile_pool(name="pstat", bufs=1, space="PSUM")

        def load(name, p, shape, dt=None):
            t = sb.tile(list(shape), dt or DT, name=name)
            nc.sync.dma_start(t[:], p[:])
            return t

        xt_sb = load("xt_sb", xt, [128, KT * TT])
        xi_sb = load("xi_sb", xi, [128, KI * TI])
        wt_sb = load("wt_sb", wt, [128, KT * D])
        wi_sb = load("wi_sb", wi, [128, KI * D])
        w1t_sb = load("w1t_sb", w1t, [128, KD * D])
        w1i_sb = load("w1i_sb", w1i, [128, KD * D])
        w2_sb = load("w2_sb", w2oh, [128, KD * BL * BL])
        btx_sb = load("btx_sb", btx, [128, KD], F32)
        bim_sb = load("bim_sb", bim, [128, KD], F32)
        gim_sb = load("gim_sb", gim, [128, KD], F32)
        beE_sb = load("beE_sb", beE, [128, KD], F32)
        b2_sb = load("b2_sb", b2c, [BL, 1], F32)
        indER_sb = load("indER_sb", indER_p, [128, ER])
        eps_sb = load("eps_sb", epsc, [1, 1], F32)
        half_sb = load("half_sb", halfc, [BL, 1], F32)
        onesc_sb = load("onesc_sb", ones_c, [128, 1])
        onesr_sb = load("onesr_sb", ones_r, [1, 128])
        id_sb = load("id_sb", ident, [128, 128])

        # HAM warmup while input DMAs run: dense dummy matmuls on the ones col
        wps = pp.tile([1, D], F32, name="wps", tag="warm", bufs=1)
        for i in range(24):
            nc.tensor.matmul(wps[:], onesc_sb[:], w2_sb[:, 0:64].broadcast_to([128, D]) if False else wt_sb[0:128, 0:D],
                             start=(i == 0), stop=(i == 23))
        warm_sb = sb.tile([1, D], F32)
        nc.scalar.copy(warm_sb[:], wps[:])
        nc.sync.dma_start(warm_out[:], warm_sb[:])

        def proj_ln(x_sb, w_sb, bias_col, kch, tok, tag):
            """x (transposed, chunked) @ W -> relu -> LN normalize (no affine).
            Returns zn [128, KD*tok] in DT."""
            z = sb.tile([128, KD * tok], DT, name=f"z_{tag}")
            for m in range(KD):
                ps = pp.tile([128, tok], F32, name=f"ps_{tag}", tag="mm")
                for k in range(kch):
                    nc.tensor.matmul(
                        ps[:],
                        w_sb[:, k * D + m * 128:k * D + (m + 1) * 128],
                        x_sb[:, k * tok:(k + 1) * tok],
                        start=(k == 0), stop=(k == kch - 1),
                    )
                nc.scalar.activation(
                    z[:, m * tok:(m + 1) * tok], ps[:], AF.Relu,
                    bias=bias_col[:, m:m + 1], scale=1.0,
                )
            zsq = sb.tile([128, KD * tok], DT, name=f"zsq_{tag}")
            nc.scalar.activation(zsq[:], z[:], AF.Square)
            s1 = pstat.tile([1, tok], F32, name=f"s1_{tag}", tag="s1")
            s2 = pstat.tile([1, tok], F32, name=f"s2_{tag}", tag="s2")
            for m in range(KD):
                nc.tensor.matmul(s1[:], onesc_sb[:], z[:, m * tok:(m + 1) * tok],
                                 start=(m == 0), stop=(m == KD - 1))
            for m in range(KD):
                nc.tensor.matmul(s2[:], onesc_sb[:], zsq[:, m * tok:(m + 1) * tok],
                                 start=(m == 0), stop=(m == KD - 1))
            mean = sb.tile([1, tok], F32, name=f"mean_{tag}")
            nc.vector.tensor_scalar_mul(mean[:], s1[:], 1.0 / D)
            msq = sb.tile([1, tok], F32, name=f"msq_{tag}")
            nc.scalar.activation(msq[:], mean[:], AF.Square)
            var = sb.tile([1, tok], F32, name=f"var_{tag}")
            nc.vector.scalar_tensor_tensor(var[:], s2[:], 1.0 / D, msq[:],
                                           op0=OP.mult, op1=OP.subtract)
            lv = sb.tile([1, tok], F32, name=f"lv_{tag}")
            nc.scalar.activation(lv[:], var[:], AF.Ln, bias=eps_sb[0:1, 0:1], scale=1.0)
            a_row = sb.tile([1, tok], DT, name=f"a_{tag}")
            nc.scalar.activation(a_row[:], lv[:], AF.Exp, bias=0.0, scale=-0.5)
            c_row = sb.tile([1, tok], DT, name=f"c_{tag}")
            nc.vector.scalar_tensor_tensor(c_row[:], mean[:], -1.0, a_row[:],
                                           op0=OP.mult, op1=OP.mult)
            psA = pstat.tile([128, tok], F32, name=f"psA_{tag}", tag="psA")
            psC = pstat.tile([128, tok], F32, name=f"psC_{tag}", tag="psC")
            nc.tensor.matmul(psA[:], onesr_sb[:], a_row[:], start=True, stop=True)
            nc.tensor.matmul(psC[:], onesr_sb[:], c_row[:], start=True, stop=True)
            zn = sb.tile([128, KD * tok], DT, name=f"zn_{tag}")
            for m in range(KD):
                tmp = sb.tile([128, tok], F32, name=f"tmp_{tag}", tag=f"tmp_{tag}", bufs=2)
                nc.vector.tensor_tensor(tmp[:], z[:, m * tok:(m + 1) * tok], psA[:],
                                        op=OP.mult)
                nc.vector.tensor_tensor(zn[:, m * tok:(m + 1) * tok], tmp[:], psC[:],
                                        op=OP.add)
            return zn

        zn_t = proj_ln(xt_sb, wt_sb, btx_sb, KT, TT, "t")
        zn_i = proj_ln(xi_sb, wi_sb, bim_sb, KI, TI, "i")

        # h_t / h_i in standard [token, dhat] layout, packed into one tile:
        # pack[:, b*D + dh]: rows 0-63 = h_t[b, e, dh], rows 64-99 = h_i[b, r, dh],
        # row 100 = bias_ht + bias_hi (added via the indicator's ones row),
        # rows 101+ zeroed (indicator rows there are zero anyway).
        pack = sb.tile([128, BL * D], DT)
        nc.vector.memset(pack[96:128, :], 0.0)
        nc.sync.dma_start(pack[100:101, :], biasrow_p[:])
        for mt in range(2):  # text token tiles (128 tokens = 2 samples each)
            ps = pp.tile([128, D], F32, name="ps_ht", tag="mm")
            for k in range(KD):
                nc.tensor.matmul(ps[:], zn_t[:, k * TT + mt * 128:k * TT + (mt + 1) * 128],
                                 w1t_sb[:, k * D:(k + 1) * D],
                                 start=(k == 0), stop=(k == KD - 1))
            for j in range(2):
                b = 2 * mt + j
                nc.vector.tensor_copy(pack[0:64, b * D:(b + 1) * D],
                                      ps[j * 64:(j + 1) * 64, :])
        for b in range(BL):
            ps = pp.tile([36, D], F32, name="ps_hi", tag="mmi", bufs=1)
            for k in range(KD):
                nc.tensor.matmul(ps[:], zn_i[:, k * TI + b * R:k * TI + (b + 1) * R],
                                 w1i_sb[:, k * D:(k + 1) * D],
                                 start=(k == 0), stop=(k == KD - 1))
            nc.vector.tensor_copy(pack[64:100, b * D:(b + 1) * D], ps[:])

        pstat.release()
        pp.release()
        psc = ctx.enter_context(tc.tile_pool(name="psc", bufs=1, space="PSUM"))
        php = ctx.enter_context(tc.tile_pool(name="php", bufs=3, space="PSUM"))
        # pairwise: H = h_t[e] + h_i[r] + bias via one indicator matmul per
        # (b, c, ntile); relu-evac from PSUM; then w2 one-hot dot into sc_ps.
        NT = [(0, 512), (512, 512), (1024, 512), (1536, 512), (2048, 256)]
        sc_ps = [psc.tile([BL, n], F32, name=f"sc{i}", tag=f"sc{i}")
                 for i, (o, n) in enumerate(NT)]
        hp_pool = ctx.enter_context(tc.tile_pool(name="hp", bufs=3))
        # software-pipelined at H-slice granularity so the dot matmul for
        # slice k never blocks the expansion matmul for slice k+1 in PE's FIFO
        slices = []
        for b in range(BL):
            for c in range(KD):
                tid = b * KD + c
                hp2 = hp_pool.tile([128, ER], DT, name="hp2", tag="hp2")
                for i, (o, n) in enumerate(NT):
                    slices.append((tid, b, c, hp2, i, o, n))
        pend = []
        for si, (tid, b, c, hp2, i, o, n) in enumerate(slices):
            hps = php.tile([128, n], F32, name="hps", tag="H")
            nc.tensor.matmul(hps[:], pack[:, b * D + c * 128:b * D + (c + 1) * 128],
                             indER_sb[:, o:o + n], start=True, stop=True)
            pend.append((tid, b, c, hp2, i, o, n, hps))
            if len(pend) >= 3:
                _emit_slice(nc, pend.pop(0), sc_ps, w2_sb, si)
        for k, item in enumerate(pend):
            _emit_slice(nc, item, sc_ps, w2_sb, k)

        tanh_sb = sb.tile([BL, ER], F32)
        for i, (o, n) in enumerate(NT):
            nc.scalar.activation(tanh_sb[:, o:o + n], sc_ps[i][:], AF.Tanh,
                                 bias=b2_sb[:, 0:1], scale=0.5)
        scores_sb = sb.tile([BL, ER], F32)
        nc.vector.tensor_scalar(scores_sb[:], tanh_sb[:], 0.5, op0=OP.mult,
                                scalar2=0.5, op1=OP.add)
        nc.sync.dma_start(scores_out[:], scores_sb[:].rearrange("b (e r) -> b e r", e=E))

        # softmax weights folded into u; exp(0.5*tanh + 0.5)
        exp_sb = sb.tile([BL, ER], DT)
        nc.scalar.activation(exp_sb[:], tanh_sb[:], AF.Exp,
                             bias=half_sb[:, 0:1], scale=0.5)
        u = sb.tile([BL, R], F32)
        nc.vector.tensor_reduce(u[:], exp_sb[:].rearrange("b (e r) -> b r e", e=E),
                                axis=mybir.AxisListType.X, op=OP.add)
        den = sb.tile([BL, 1], F32)
        nc.vector.tensor_reduce(den[:], u[:], axis=mybir.AxisListType.X, op=OP.add)
        rden = sb.tile([BL, 1], F32)
        nc.vector.reciprocal(rden[:], den[:])
        u_f = sb.tile([BL, R], DT)
        nc.vector.tensor_scalar(u_f[:], u[:], rden[:, 0:1], op0=OP.mult,
                                scalar2=1.0 / E, op1=OP.mult)
        u_row = sb.tile([1, TI], DT)
        nc.sync.dma_start(u_row[0:1, :].rearrange("q (b r) -> q b r", b=BL), u_f[:])

        psU = php.tile([128, TI], F32, tag="H")
        nc.tensor.matmul(psU[:], onesr_sb[:], u_row[:], start=True, stop=True)
        aggT = sb.tile([128, KD * BL], F32)
        for c in range(KD):
            tmp = sb.tile([128, TI], F32, name="agg_tmp", tag="agg_tmp", bufs=2)
            nc.vector.tensor_tensor(tmp[:], zn_i[:, c * TI:(c + 1) * TI], psU[:],
                                    op=OP.mult)
            nc.vector.tensor_reduce(
                aggT[:, c * BL:(c + 1) * BL],
                tmp[:].rearrange("p (b r) -> p b r", b=BL),
                axis=mybir.AxisListType.X, op=OP.add)
        aggF = sb.tile([128, KD * BL], DT)
        for c in range(KD):
            nc.vector.scalar_tensor_tensor(
                aggF[:, c * BL:(c + 1) * BL], aggT[:, c * BL:(c + 1) * BL],
                gim_sb[:, c:c + 1],
                beE_sb[:, c:c + 1].broadcast_to([128, BL]),
                op0=OP.mult, op1=OP.add)
        psT = php.tile([KD * BL, 128], DT, tag="H")
        nc.tensor.transpose(psT[:], aggF[:], id_sb[:])
        agg_sb = sb.tile([KD * BL, 128], F32)
        nc.scalar.copy(agg_sb[:], psT[:])
        # row (c*BL + b) -> agg_out[b, c*128 : (c+1)*128]
        for c in range(KD):
            nc.sync.dma_start(agg_out[:, c * 128:(c + 1) * 128],
                              agg_sb[c * BL:(c + 1) * BL, :])

    nc.compile()
    return nc


def _fold(a, nchunk):
    """[nchunk*128, cols] -> [128, nchunk*cols] partition-major layout."""
    n, cols = a.shape
    assert n == nchunk * 128
    return np.ascontiguousarray(
        a.reshape(nchunk, 128, cols).transpose(1, 0, 2).reshape(128, nchunk * cols))


def _prep_host(inputs):
    npdt = _np_dt()
    f32 = np.float32

    text = np.asarray(inputs["text_feats"], f32)
    image = np.asarray(inputs["image_feats"], f32)
    W_text = np.asarray(inputs["W_text"], f32)
    b_text = np.asarray(inputs["b_text"], f32)
    g_text = np.asarray(inputs["g_text"], f32)
    beta_text = np.asarray(inputs["beta_text"], f32)
    W_img = np.asarray(inputs["W_img"], f32)
    b_img = np.asarray(inputs["b_img"], f32)
    g_img = np.asarray(inputs["g_img"], f32)
    beta_img = np.asarray(inputs["beta_img"], f32)
    W1 = np.asarray(inputs["W1"], f32)
    b1 = np.asarray(inputs["b1"], f32)
    W2 = np.asarray(inputs["W2"], f32)
    b2 = np.asarray(inputs["b2"], f32)

    W1t = W1[:D] * g_text[:, None]
    W1i = W1[D:] * g_img[:, None]
    bht = beta_text @ W1[:D]
    bhi = beta_img @ W1[D:] + b1

    shared = {
        "wt": _fold(W_text, KT).astype(npdt),
        "wi": _fold(W_img, KI).astype(npdt),
        "w1t": _fold(W1t, KD).astype(npdt),
        "w1i": _fold(W1i, KD).astype(npdt),
        "btx": np.ascontiguousarray(b_text.reshape(KD, 128).T),
        "bim": np.ascontiguousarray(b_img.reshape(KD, 128).T),
        "biasrow": np.tile((bht + bhi), BL).reshape(1, BL * D).astype(npdt),
        "gim": np.ascontiguousarray(g_img.reshape(KD, 128).T),
        "beE": np.ascontiguousarray((beta_img / E).reshape(KD, 128).T),
        "b2c": np.full((BL, 1), b2[0] * 0.5, f32),
        "halfc": np.full((BL, 1), 0.5, f32),
        "epsc": np.full((1, 1), LN_EPS, f32),
        "ones_c": np.ones((128, 1), f32).astype(npdt),
        "ones_r": np.ones((1, 128), f32).astype(npdt),
        "ident": np.eye(128, dtype=f32).astype(npdt),
    }
    # one-hot w2: [128, (c, b, col)] where col b holds w2 chunk c
    w2f = W2[:, 0].reshape(KD, 128)
    w2oh = np.zeros((128, KD, BL, BL), f32)
    for c in range(KD):
        for b in range(BL):
            w2oh[:, c, b, b] = w2f[c]
    shared["w2oh"] = w2oh.reshape(128, KD * BL * BL).astype(npdt)
    ind = np.zeros((128, E, R), f32)
    for e in range(E):
        ind[e, e, :] = 1.0
    for r in range(R):
        ind[64 + r, :, r] = 1.0
    ind[100, :, :] = 1.0
    shared["indER"] = ind.reshape(128, ER).astype(npdt)

    in_maps = []
    for core in range(NCORES):
        sl = slice(core * BL, (core + 1) * BL)
        xt = _fold(np.ascontiguousarray(text[sl].reshape(TT, Dt).T), KT)
        xi = _fold(np.ascontiguousarray(image[sl].reshape(TI, Di).T), KI)
        m = dict(shared)
        m["xt"] = xt.astype(npdt)
        m["xi"] = xi.astype(npdt)
        in_maps.append(m)
    return in_maps


def _install_ntff_hook():
    """The slim container lacks antenv.axon_hooks; recreate it so
    run_bass_kernel_spmd(trace=True) can capture NTFF profiles."""
    import types, ctypes, contextlib

    try:
        from antenv.axon_hooks import get_axon_ntff_profile_hook  # noqa: F401
        return
    except ImportError:
        pass
    so_path = "/opt/axon/libaxon_pjrt.so"
    try:
        lib = ctypes.CDLL(so_path)
    except OSError:
        return
    if not hasattr(lib, "axon_start_nrt_profile"):
        return
    lib.axon_start_nrt_profile.argtypes = [ctypes.POINTER(ctypes.c_int64), ctypes.c_size_t]
    lib.axon_start_nrt_profile.restype = ctypes.c_int64
    lib.axon_stop_nrt_profile.argtypes = [ctypes.c_char_p]
    lib.axon_stop_nrt_profile.restype = ctypes.c_int64

    @contextlib.contextmanager
    def _hook(output_dir, device_ids):
        import jax
        jax.devices()
        if device_ids:
            ids = (ctypes.c_int64 * len(device_ids))(*device_ids)
            rc = lib.axon_start_nrt_profile(ids, len(device_ids))
        else:
            rc = lib.axon_start_nrt_profile(None, 0)
        if rc != 0:
            raise RuntimeError(f"axon_start_nrt_profile rc={rc}")
        try:
            yield
        finally:
            n = lib.axon_stop_nrt_profile(str(output_dir).encode())
            print(f"ntff profile: {n} file(s) -> {output_dir}", file=sys.stderr)

    mod = types.ModuleType("antenv.axon_hooks")
    mod.get_axon_ntff_profile_hook = lambda: _hook
    mod.set_axon_ntff_profile_hook = lambda h: None
    sys.modules["antenv.axon_hooks"] = mod
    import antenv
    antenv.axon_hooks = mod


def _get_program():
    key = DT_MODE
    if key not in _CACHE:
        _CACHE[key] = _build_program()
    return _CACHE[key]


def kernel(trace=False, **inputs):
    if trace:
        _install_ntff_hook()
    nc = _get_program()
    in_maps = _prep_host(inputs)
    res = run_bass_kernel_spmd(nc, in_maps, list(range(NCORES)), trace=trace)
    scores = np.concatenate([res.results[c]["scores_out"] for c in range(NCORES)], axis=0)
    agg = np.concatenate([res.results[c]["agg_out"] for c in range(NCORES)], axis=0)
    out = (scores.astype(np.float32), agg.astype(np.float32))
    if trace:
        return out, res
    return out


# revision 19
# speedup vs baseline: 1.1968x; 1.0864x over previous
"""Trainium2 Bass kernel for nn_EntityAlignmentModule.

Data-parallel over batch B=32 across 8 NeuronCores (4 samples/core).
All activations kept in transposed [feature, token] layout on chip;
LayerNorm statistics are computed with ones-vector matmuls on the
TensorEngine (partition-dim sums), gamma/beta are folded into the
following matmul's weights on the host, and the pairwise
relu(h_t[:,e] + h_i[:,r]) stage runs as broadcast-AP adds on the
Vector/GpSimd engines with the w2 contraction done on the TensorEngine
via per-batch one-hot weight columns accumulating into one [4, N] PSUM.
"""

import sys
import numpy as np

sys.path.insert(0, "/opt/trn_rl_repo")

import ml_dtypes  # noqa: E402
import concourse.bass as bass  # noqa: E402
import concourse.tile as tile  # noqa: E402
from concourse import bacc, mybir  # noqa: E402
from concourse.bass_utils import run_bass_kernel_spmd  # noqa: E402
from contextlib import ExitStack  # noqa: E402

AF = mybir.ActivationFunctionType
OP = mybir.AluOpType
F32 = mybir.dt.float32

B, E, R = 32, 64, 36
Dt, Di, D = 768, 2048, 512
LN_EPS = 1e-5
NCORES = 8
BL = B // NCORES          # 4 samples per core
TT = BL * E               # 256 text tokens per core
TI = BL * R               # 144 image tokens per core
KT = Dt // 128            # 6 text K chunks
KI = Di // 128            # 16 image K chunks
KD = D // 128             # 4 chunks of D
ER = E * R                # 2304 pairs per sample

# dtype mode for matmul operands: "bf16" (fast, ~5e-3 err) or
# "f32r" (TF32-like, ~2e-4 err, 2x DMA traffic)
DT_MODE = "bf16"
# pairwise tile ids (b*KD+c): adds on GpSimd for GP_TILES, else DVE;
# relus on DVE for DVE_RELU_TILES, else ACT
GP_TILES = tuple(int(x) for x in __import__("os").environ.get("KGP", "1,4,7,10,13").split(",") if x != "")
DVE_RELU_TILES = tuple(int(x) for x in __import__("os").environ.get("KDR", "0,2,4,6,8,10,12,14").split(",") if x != "")

_CACHE: dict = {}


def _register_relu_add():
    """Register a fused out = relu(in0 + in1) custom DVE op."""
    from concourse import dve_ops
    from concourse.dve_spec import Spec, Src0, Src1, relu, lower
    from concourse.dve_uop import DveOpSpec

    name = "RELU_ADD_KRN"
    if name in dve_ops._SUB_OPCODE_FOR_NAME:
        return next(op for op in dve_ops.OPS if op.name == name)
    spec = Spec(
        body=relu(Src0 + Src1),
        reference=lambda in0, in1, s0, s1, imm2: np.maximum(
            np.nan_to_num(in0.astype(np.float32) + in1), 0),
    )
    row = dve_ops._CUSTOM_DVE_ROW_BASE + len(dve_ops.OPS)
    assert row < 0x20
    shas = {}
    for ver in ("v3", "v4"):
        try:
            uops = lower(spec, ver=ver)
            shas[ver] = DveOpSpec(name=name, opcode=row, uops=uops,
                                  rd1_en=True).sha(ver)
        except Exception:
            pass
    op = dve_ops.DveOp(name, spec, subdim=False, uops_sha=shas)
    dve_ops.OPS.append(op)
    dve_ops.CUSTOM_DVE_SPECS[name] = spec
    dve_ops._SUB_OPCODE_FOR_NAME[name] = row
    return op


_RELU_ADD = _register_relu_add()


def _emit_slice(nc, item, sc_ps, w2_sb, si):
    tid, b, c, hp2, i, o, n, hps = item
    last = (tid == BL * KD - 1)
    if si % 2 == 0:
        nc.vector.tensor_scalar_max(hp2[:, o:o + n], hps[:], 0.0)
    else:
        nc.scalar.activation(hp2[:, o:o + n], hps[:], AF.Relu)
    nc.tensor.matmul(sc_ps[i][:], w2_sb[:, (c * BL + b) * BL:(c * BL + b + 1) * BL],
                     hp2[:, o:o + n], start=(tid == 0), stop=last)


def _np_dt():
    return ml_dtypes.bfloat16 if DT_MODE == "bf16" else np.float32


def _bir_dt():
    return mybir.dt.bfloat16 if DT_MODE == "bf16" else mybir.dt.float32r


def _build_program():
    DT = _bir_dt()
    nc = bacc.Bacc()

    def par(name, shape, dt=None):
        return nc.declare_dram_parameter(name, list(shape), dt or DT, isOutput=False)

    # per-core activations, [128, chunks*tok] partition-major folded layouts
    xt = par("xt", [128, KT * TT])
    xi = par("xi", [128, KI * TI])
    # weights, [128, chunks*512]
    wt = par("wt", [128, KT * D])
    wi = par("wi", [128, KI * D])
    w1t = par("w1t", [128, KD * D])
    w1i = par("w1i", [128, KD * D])
    # one-hot w2 columns: [128, (c, b, 4)]
    w2oh = par("w2oh", [128, KD * BL * BL])
    # per-partition bias/scale columns (f32)
    btx = par("btx", [128, KD], F32)
    bim = par("bim", [128, KD], F32)
    gim = par("gim", [128, KD], F32)
    beE = par("beE", [128, KD], F32)
    b2c = par("b2c", [BL, 1], F32)
    biasrow_p = par("biasrow", [1, BL * D])
    indER_p = par("indER", [128, ER])
    epsc = par("epsc", [1, 1], F32)
    halfc = par("halfc", [BL, 1], F32)
    ones_c = par("ones_c", [128, 1])
    ones_r = par("ones_r", [1, 128])
    ident = par("ident", [128, 128])

    warm_out = nc.declare_dram_parameter("warm_out", [1, D], F32, isOutput=True)
    scores_out = nc.declare_dram_parameter("scores_out", [BL, E, R], F32, isOutput=True)
    agg_out = nc.declare_dram_parameter("agg_out", [BL, D], F32, isOutput=True)

    with ExitStack() as ctx:
        tc = tile.TileContext(nc)
        ctx.enter_context(tc)
        sb = ctx.enter_context(tc.tile_pool(name="sb", bufs=1))
        pp = tc.alloc_tile_pool(name="pp", bufs=2, space="PSUM")
        pstat = tc.alloc_tile_pool(name="pstat", bufs=1, space="PSUM")

        def load(name, p, shape, dt=None):
            t = sb.tile(list(shape), dt or DT, name=name)
            nc.sync.dma_start(t[:], p[:])
            return t

        onesc_sb = load("onesc_sb", ones_c, [128, 1])
        onesr_sb = load("onesr_sb", ones_r, [1, 128])
        btx_sb = load("btx_sb", btx, [128, KD], F32)
        bim_sb = load("bim_sb", bim, [128, KD], F32)
        gim_sb = load("gim_sb", gim, [128, KD], F32)
        beE_sb = load("beE_sb", beE, [128, KD], F32)
        b2_sb = load("b2_sb", b2c, [BL, 1], F32)
        eps_sb = load("eps_sb", epsc, [1, 1], F32)
        half_sb = load("half_sb", halfc, [BL, 1], F32)
        id_sb = load("id_sb", ident, [128, 128])
        xt_sb = load("xt_sb", xt, [128, KT * TT])
        xi_sb = load("xi_sb", xi, [128, KI * TI])
        wt_sb = load("wt_sb", wt, [128, KT * D])
        wi_sb = load("wi_sb", wi, [128, KI * D])
        w1t_sb = load("w1t_sb", w1t, [128, KD * D])
        w1i_sb = load("w1i_sb", w1i, [128, KD * D])
        w2_sb = load("w2_sb", w2oh, [128, KD * BL * BL])
        indER_sb = load("indER_sb", indER_p, [128, ER])

        # HAM warmup while input DMAs run: dense dummy matmuls
        wps = pp.tile([1, D], F32, name="wps", tag="mmi", bufs=1)
        for i in range(12):
            nc.tensor.matmul(wps[:], onesc_sb[:], xt_sb[0:128, 0:D],
                             start=(i == 0), stop=(i == 11))
        warm_sb = sb.tile([1, D], F32)
        nc.scalar.copy(warm_sb[:], wps[:])
        nc.sync.dma_start(warm_out[:], warm_sb[:])

        def proj_phase(x_sb, w_sb, bias_col, kch, tok, tag):
            """x (transposed, chunked) @ W -> relu -> z; stat sums on PE."""
            z = sb.tile([128, KD * tok], DT, name=f"z_{tag}")
            for m in range(KD):
                ps = pp.tile([128, tok], F32, name=f"ps_{tag}", tag="mm")
                for k in range(kch):
                    nc.tensor.matmul(
                        ps[:],
                        w_sb[:, k * D + m * 128:k * D + (m + 1) * 128],
                        x_sb[:, k * tok:(k + 1) * tok],
                        start=(k == 0), stop=(k == kch - 1),
                    )
                nc.scalar.activation(
                    z[:, m * tok:(m + 1) * tok], ps[:], AF.Relu,
                    bias=bias_col[:, m:m + 1], scale=1.0,
                )
            zsq = sb.tile([128, KD * tok], DT, name=f"zsq_{tag}")
            for m in range(KD):
                sl = slice(m * tok, (m + 1) * tok)
                nc.vector.tensor_tensor(zsq[:, sl], z[:, sl], z[:, sl], op=OP.mult)
            s12 = pstat.tile([1, 2 * tok], F32, name=f"s12_{tag}", tag=f"s12_{tag}")
            s1 = s12[:, 0:tok]
            s2 = s12[:, tok:2 * tok]
            for m in range(KD):
                nc.tensor.matmul(s1, onesc_sb[:], z[:, m * tok:(m + 1) * tok],
                                 start=(m == 0), stop=(m == KD - 1))
            for m in range(KD):
                nc.tensor.matmul(s2, onesc_sb[:], zsq[:, m * tok:(m + 1) * tok],
                                 start=(m == 0), stop=(m == KD - 1))
            return z, s1, s2

        def ln_var(s1, s2, tok, tag):
            msq = sb.tile([1, tok], F32, name=f"msq_{tag}")
            nc.scalar.activation(msq[:], s1, AF.Square, bias=0.0, scale=1.0 / D)
            var = sb.tile([1, tok], F32, name=f"var_{tag}")
            nc.vector.scalar_tensor_tensor(var[:], s2, 1.0 / D, msq[:],
                                           op0=OP.mult, op1=OP.subtract)
            return var

        def ln_norm(z, s1, a_row, tok, tag):
            c_row = sb.tile([1, tok], DT, name=f"c_{tag}")
            nc.vector.scalar_tensor_tensor(c_row[:], s1, -1.0 / D, a_row[:],
                                           op0=OP.mult, op1=OP.mult)
            psA = pstat.tile([128, tok], F32, name=f"psA_{tag}", tag="psA")
            psC = pstat.tile([128, tok], F32, name=f"psC_{tag}", tag="psC")
            nc.tensor.matmul(psA[:], onesr_sb[:], a_row[:], start=True, stop=True)
            nc.tensor.matmul(psC[:], onesr_sb[:], c_row[:], start=True, stop=True)
            zn = sb.tile([128, KD * tok], DT, name=f"zn_{tag}")
            for m in range(KD):
                tmp = sb.tile([128, tok], F32, name=f"tmp_{tag}", tag=f"tmp_{tag}", bufs=2)
                nc.vector.tensor_tensor(tmp[:], z[:, m * tok:(m + 1) * tok], psA[:],
                                        op=OP.mult)
                nc.vector.tensor_tensor(zn[:, m * tok:(m + 1) * tok], tmp[:], psC[:],
                                        op=OP.add)
            return zn

        z_t, s1_t, s2_t = proj_phase(xt_sb, wt_sb, btx_sb, KT, TT, "t")
        z_i, s1_i, s2_i = proj_phase(xi_sb, wi_sb, bim_sb, KI, TI, "i")
        var_t = ln_var(s1_t, s2_t, TT, "t")
        var_i = ln_var(s1_i, s2_i, TI, "i")
        # batch Ln/Ln then Exp/Exp so the ACT table set switches only once
        lv_t = sb.tile([1, TT], F32)
        lv_i = sb.tile([1, TI], F32)
        nc.scalar.activation(lv_t[:], var_t[:], AF.Ln, bias=eps_sb[0:1, 0:1], scale=1.0)
        nc.scalar.activation(lv_i[:], var_i[:], AF.Ln, bias=eps_sb[0:1, 0:1], scale=1.0)
        a_t = sb.tile([1, TT], DT)
        a_i = sb.tile([1, TI], DT)
        nc.scalar.activation(a_t[:], lv_t[:], AF.Exp, bias=0.0, scale=-0.5)
        nc.scalar.activation(a_i[:], lv_i[:], AF.Exp, bias=0.0, scale=-0.5)
        zn_t = ln_norm(z_t, s1_t, a_t, TT, "t")
        zn_i = ln_norm(z_i, s1_i, a_i, TI, "i")

        # h_t / h_i in standard [token, dhat] layout, packed into one tile:
        # pack[:, b*D + dh]: rows 0-63 = h_t[b, e, dh], rows 64-99 = h_i[b, r, dh],
        # row 100 = bias_ht + bias_hi (added via the indicator's ones row),
        # rows 101+ zeroed (indicator rows there are zero anyway).
        pack = sb.tile([128, BL * D], DT)
        nc.vector.memset(pack[96:128, :], 0.0)
        nc.sync.dma_start(pack[100:101, :], biasrow_p[:])
        for mt in range(2):  # text token tiles (128 tokens = 2 samples each)
            ps = pp.tile([128, D], F32, name="ps_ht", tag="mm")
            for k in range(KD):
                nc.tensor.matmul(ps[:], zn_t[:, k * TT + mt * 128:k * TT + (mt + 1) * 128],
                                 w1t_sb[:, k * D:(k + 1) * D],
                                 start=(k == 0), stop=(k == KD - 1))
            for j in range(2):
                b = 2 * mt + j
                nc.vector.tensor_copy(pack[0:64, b * D:(b + 1) * D],
                                      ps[j * 64:(j + 1) * 64, :])
        for b in range(BL):
            ps = pp.tile([36, D], F32, name="ps_hi", tag="mmi", bufs=1)
            for k in range(KD):
                nc.tensor.matmul(ps[:], zn_i[:, k * TI + b * R:k * TI + (b + 1) * R],
                                 w1i_sb[:, k * D:(k + 1) * D],
                                 start=(k == 0), stop=(k == KD - 1))
            nc.vector.tensor_copy(pack[64:100, b * D:(b + 1) * D], ps[:])

        pstat.release()
        pp.release()
        psc = ctx.enter_context(tc.tile_pool(name="psc", bufs=1, space="PSUM"))
        php = ctx.enter_context(tc.tile_pool(name="php", bufs=3, space="PSUM"))
        # pairwise: H = h_t[e] + h_i[r] + bias via one indicator matmul per
        # (b, c, ntile); relu-evac from PSUM; then w2 one-hot dot into sc_ps.
        NT = [(0, 512), (512, 512), (1024, 512), (1536, 512), (2048, 256)]
        sc_ps = [psc.tile([BL, n], F32, name=f"sc{i}", tag=f"sc{i}")
                 for i, (o, n) in enumerate(NT)]
        hp_pool = ctx.enter_context(tc.tile_pool(name="hp", bufs=3))
        # software-pipelined at H-slice granularity so the dot matmul for
        # slice k never blocks the expansion matmul for slice k+1 in PE's FIFO
        slices = []
        for b in range(BL):
            for c in range(KD):
                tid = b * KD + c
                hp2 = hp_pool.tile([128, ER], DT, name="hp2", tag="hp2")
                for i, (o, n) in enumerate(NT):
                    slices.append((tid, b, c, hp2, i, o, n))
        pend = []
        for si, (tid, b, c, hp2, i, o, n) in enumerate(slices):
            hps = php.tile([128, n], F32, name="hps", tag="H")
            nc.tensor.matmul(hps[:], pack[:, b * D + c * 128:b * D + (c + 1) * 128],
                             indER_sb[:, o:o + n], start=True, stop=True)
            pend.append((tid, b, c, hp2, i, o, n, hps))
            if len(pend) >= 3:
                _emit_slice(nc, pend.pop(0), sc_ps, w2_sb, si)
        for k, item in enumerate(pend):
            _emit_slice(nc, item, sc_ps, w2_sb, k)

        tanh_sb = sb.tile([BL, ER], F32)
        for i, (o, n) in enumerate(NT):
            nc.scalar.activation(tanh_sb[:, o:o + n], sc_ps[i][:], AF.Tanh,
                                 bias=b2_sb[:, 0:1], scale=0.5)
        scores_sb = sb.tile([BL, ER], F32)
        nc.vector.tensor_scalar(scores_sb[:], tanh_sb[:], 0.5, op0=OP.mult,
                                scalar2=0.5, op1=OP.add)
        nc.sync.dma_start(scores_out[:], scores_sb[:].rearrange("b (e r) -> b e r", e=E))

        # softmax weights folded into u; exp(0.5*tanh + 0.5)
        exp_sb = sb.tile([BL, ER], DT)
        nc.scalar.activation(exp_sb[:], tanh_sb[:], AF.Exp,
                             bias=half_sb[:, 0:1], scale=0.5)
        u = sb.tile([BL, R], F32)
        nc.vector.tensor_reduce(u[:], exp_sb[:].rearrange("b (e r) -> b r e", e=E),
                                axis=mybir.AxisListType.X, op=OP.add)
        den = sb.tile([BL, 1], F32)
        nc.vector.tensor_reduce(den[:], u[:], axis=mybir.AxisListType.X, op=OP.add)
        rden = sb.tile([BL, 1], F32)
        nc.vector.reciprocal(rden[:], den[:])
        u_f = sb.tile([BL, R], DT)
        nc.vector.tensor_scalar(u_f[:], u[:], rden[:, 0:1], op0=OP.mult,
                                scalar2=1.0 / E, op1=OP.mult)
        u_row = sb.tile([1, TI], DT)
        nc.sync.dma_start(u_row[0:1, :].rearrange("q (b r) -> q b r", b=BL), u_f[:])

        psU = php.tile([128, TI], F32, tag="H")
        nc.tensor.matmul(psU[:], onesr_sb[:], u_row[:], start=True, stop=True)
        aggT = sb.tile([128, KD * BL], F32)
        for c in range(KD):
            tmp = sb.tile([128, TI], F32, name="agg_tmp", tag="agg_tmp", bufs=2)
            nc.vector.tensor_tensor(tmp[:], zn_i[:, c * TI:(c + 1) * TI], psU[:],
                                    op=OP.mult)
            nc.vector.tensor_reduce(
                aggT[:, c * BL:(c + 1) * BL],
                tmp[:].rearrange("p (b r) -> p b r", b=BL),
                axis=mybir.AxisListType.X, op=OP.add)
        aggF = sb.tile([128, KD * BL], DT)
        for c in range(KD):
            nc.vector.scalar_tensor_tensor(
                aggF[:, c * BL:(c + 1) * BL], aggT[:, c * BL:(c + 1) * BL],
                gim_sb[:, c:c + 1],
                beE_sb[:, c:c + 1].broadcast_to([128, BL]),
                op0=OP.mult, op1=OP.add)
        psT = php.tile([KD * BL, 128], DT, tag="H")
        nc.tensor.transpose(psT[:], aggF[:], id_sb[:])
        agg_sb = sb.tile([KD * BL, 128], F32)
        nc.scalar.copy(agg_sb[:], psT[:])
        # row (c*BL + b) -> agg_out[b, c*128 : (c+1)*128]
        for c in range(KD):
            nc.sync.dma_start(agg_out[:, c * 128:(c + 1) * 128],
                              agg_sb[c * BL:(c + 1) * BL, :])

    nc.compile()
    return nc


def _fold(a, nchunk):
    """[nchunk*128, cols] -> [128, nchunk*cols] partition-major layout."""
    n, cols = a.shape
    assert n == nchunk * 128
    return np.ascontiguousarray(
        a.reshape(nchunk, 128, cols).transpose(1, 0, 2).reshape(128, nchunk * cols))


def _prep_host(inputs):
    npdt = _np_dt()
    f32 = np.float32

    text = np.asarray(inputs["text_feats"], f32)
    image = np.asarray(inputs["image_feats"], f32)
    W_text = np.asarray(inputs["W_text"], f32)
    b_text = np.asarray(inputs["b_text"], f32)
    g_text = np.asarray(inputs["g_text"], f32)
    beta_text = np.asarray(inputs["beta_text"], f32)
    W_img = np.asarray(inputs["W_img"], f32)
    b_img = np.asarray(inputs["b_img"], f32)
    g_img = np.asarray(inputs["g_img"], f32)
    beta_img = np.asarray(inputs["beta_img"], f32)
    W1 = np.asarray(inputs["W1"], f32)
    b1 = np.asarray(inputs["b1"], f32)
    W2 = np.asarray(inputs["W2"], f32)
    b2 = np.asarray(inputs["b2"], f32)

    W1t = W1[:D] * g_text[:, None]
    W1i = W1[D:] * g_img[:, None]
    bht = beta_text @ W1[:D]
    bhi = beta_img @ W1[D:] + b1

    shared = {
        "wt": _fold(W_text, KT).astype(npdt),
        "wi": _fold(W_img, KI).astype(npdt),
        "w1t": _fold(W1t, KD).astype(npdt),
        "w1i": _fold(W1i, KD).astype(npdt),
        "btx": np.ascontiguousarray(b_text.reshape(KD, 128).T),
        "bim": np.ascontiguousarray(b_img.reshape(KD, 128).T),
        "biasrow": np.tile((bht + bhi), BL).reshape(1, BL * D).astype(npdt),
        "gim": np.ascontiguousarray(g_img.reshape(KD, 128).T),
        "beE": np.ascontiguousarray((beta_img / E).reshape(KD, 128).T),
        "b2c": np.full((BL, 1), b2[0] * 0.5, f32),
        "halfc": np.full((BL, 1), 0.5, f32),
        "epsc": np.full((1, 1), LN_EPS, f32),
        "ones_c": np.ones((128, 1), f32).astype(npdt),
        "ones_r": np.ones((1, 128), f32).astype(npdt),
        "ident": np.eye(128, dtype=f32).astype(npdt),
    }
    # one-hot w2: [128, (c, b, col)] where col b holds w2 chunk c
    w2f = W2[:, 0].reshape(KD, 128)
    w2oh = np.zeros((128, KD, BL, BL), f32)
    for c in range(KD):
        for b in range(BL):
            w2oh[:, c, b, b] = w2f[c]
    shared["w2oh"] = w2oh.reshape(128, KD * BL * BL).astype(npdt)
    ind = np.zeros((128, E, R), f32)
    for e in range(E):
        ind[e, e, :] = 1.0
    for r in range(R):
        ind[64 + r, :, r] = 1.0
    ind[100, :, :] = 1.0
    shared["indER"] = ind.reshape(128, ER).astype(npdt)

    in_maps = []
    for core in range(NCORES):
        sl = slice(core * BL, (core + 1) * BL)
        xt = _fold(np.ascontiguousarray(text[sl].reshape(TT, Dt).T), KT)
        xi = _fold(np.ascontiguousarray(image[sl].reshape(TI, Di).T), KI)
        m = dict(shared)
        m["xt"] = xt.astype(npdt)
        m["xi"] = xi.astype(npdt)
        in_maps.append(m)
    return in_maps


def _install_ntff_hook():
    """The slim container lacks antenv.axon_hooks; recreate it so
    run_bass_kernel_spmd(trace=True) can capture NTFF profiles."""
    import types, ctypes, contextlib

    try:
        from antenv.axon_hooks import get_axon_ntff_profile_hook  # noqa: F401
        return
    except ImportError:
        pass
    so_path = "/opt/axon/libaxon_pjrt.so"
    try:
        lib = ctypes.CDLL(so_path)
    except OSError:
        return
    if not hasattr(lib, "axon_start_nrt_profile"):
        return
    lib.axon_start_nrt_profile.argtypes = [ctypes.POINTER(ctypes.c_int64), ctypes.c_size_t]
    lib.axon_start_nrt_profile.restype = ctypes.c_int64
    lib.axon_stop_nrt_profile.argtypes = [ctypes.c_char_p]
    lib.axon_stop_nrt_profile.restype = ctypes.c_int64

    @contextlib.contextmanager
    def _hook(output_dir, device_ids):
        import jax
        jax.devices()
        if device_ids:
            ids = (ctypes.c_int64 * len(device_ids))(*device_ids)
            rc = lib.axon_start_nrt_profile(ids, len(device_ids))
        else:
            rc = lib.axon_start_nrt_profile(None, 0)
        if rc != 0:
            raise RuntimeError(f"axon_start_nrt_profile rc={rc}")
        try:
            yield
        finally:
            n = lib.axon_stop_nrt_profile(str(output_dir).encode())
            print(f"ntff profile: {n} file(s) -> {output_dir}", file=sys.stderr)

    mod = types.ModuleType("antenv.axon_hooks")
    mod.get_axon_ntff_profile_hook = lambda: _hook
    mod.set_axon_ntff_profile_hook = lambda h: None
    sys.modules["antenv.axon_hooks"] = mod
    import antenv
    antenv.axon_hooks = mod


def _get_program():
    key = DT_MODE
    if key not in _CACHE:
        _CACHE[key] = _build_program()
    return _CACHE[key]


def kernel(trace=False, **inputs):
    if trace:
        _install_ntff_hook()
    nc = _get_program()
    in_maps = _prep_host(inputs)
    res = run_bass_kernel_spmd(nc, in_maps, list(range(NCORES)), trace=trace)
    scores = np.concatenate([res.results[c]["scores_out"] for c in range(NCORES)], axis=0)
    agg = np.concatenate([res.results[c]["agg_out"] for c in range(NCORES)], axis=0)
    out = (scores.astype(np.float32), agg.astype(np.float32))
    if trace:
        return out, res
    return out


# revision 22
# speedup vs baseline: 1.2377x; 1.0342x over previous
"""Trainium2 Bass kernel for nn_EntityAlignmentModule.

Data-parallel over batch B=32 across 8 NeuronCores (4 samples/core).
All activations kept in transposed [feature, token] layout on chip;
LayerNorm statistics are computed with ones-vector matmuls on the
TensorEngine (partition-dim sums), gamma/beta are folded into the
following matmul's weights on the host, and the pairwise
relu(h_t[:,e] + h_i[:,r]) stage runs as broadcast-AP adds on the
Vector/GpSimd engines with the w2 contraction done on the TensorEngine
via per-batch one-hot weight columns accumulating into one [4, N] PSUM.
"""

import sys
import numpy as np

sys.path.insert(0, "/opt/trn_rl_repo")

import ml_dtypes  # noqa: E402
import concourse.bass as bass  # noqa: E402
import concourse.tile as tile  # noqa: E402
from concourse import bacc, mybir  # noqa: E402
from concourse.bass_utils import run_bass_kernel_spmd  # noqa: E402
from contextlib import ExitStack  # noqa: E402

AF = mybir.ActivationFunctionType
OP = mybir.AluOpType
F32 = mybir.dt.float32

B, E, R = 32, 64, 36
Dt, Di, D = 768, 2048, 512
LN_EPS = 1e-5
NCORES = 8
BL = B // NCORES          # 4 samples per core
TT = BL * E               # 256 text tokens per core
TI = BL * R               # 144 image tokens per core
KT = Dt // 128            # 6 text K chunks
KI = Di // 128            # 16 image K chunks
KD = D // 128             # 4 chunks of D
ER = E * R                # 2304 pairs per sample

# dtype mode for matmul operands: "bf16" (fast, ~5e-3 err) or
# "f32r" (TF32-like, ~2e-4 err, 2x DMA traffic)
DT_MODE = "bf16"
# pairwise tile ids (b*KD+c): adds on GpSimd for GP_TILES, else DVE;
# relus on DVE for DVE_RELU_TILES, else ACT
GP_TILES = tuple(int(x) for x in __import__("os").environ.get("KGP", "1,4,7,10,13").split(",") if x != "")
DVE_RELU_TILES = tuple(int(x) for x in __import__("os").environ.get("KDR", "0,2,4,6,8,10,12,14").split(",") if x != "")

_CACHE: dict = {}


def _register_relu_add():
    """Register a fused out = relu(in0 + in1) custom DVE op."""
    from concourse import dve_ops
    from concourse.dve_spec import Spec, Src0, Src1, relu, lower
    from concourse.dve_uop import DveOpSpec

    name = "RELU_ADD_KRN"
    if name in dve_ops._SUB_OPCODE_FOR_NAME:
        return next(op for op in dve_ops.OPS if op.name == name)
    spec = Spec(
        body=relu(Src0 + Src1),
        reference=lambda in0, in1, s0, s1, imm2: np.maximum(
            np.nan_to_num(in0.astype(np.float32) + in1), 0),
    )
    row = dve_ops._CUSTOM_DVE_ROW_BASE + len(dve_ops.OPS)
    assert row < 0x20
    shas = {}
    for ver in ("v3", "v4"):
        try:
            uops = lower(spec, ver=ver)
            shas[ver] = DveOpSpec(name=name, opcode=row, uops=uops,
                                  rd1_en=True).sha(ver)
        except Exception:
            pass
    op = dve_ops.DveOp(name, spec, subdim=False, uops_sha=shas)
    dve_ops.OPS.append(op)
    dve_ops.CUSTOM_DVE_SPECS[name] = spec
    dve_ops._SUB_OPCODE_FOR_NAME[name] = row
    return op


_RELU_ADD = _register_relu_add()


def _emit_slice(nc, item, sc_ps, w2_sb, si):
    tid, b, c, hp2, i, o, n, hps = item
    last = (tid == BL * KD - 1)
    if si % 2 == 0:
        nc.vector.tensor_scalar_max(hp2[:, o:o + n], hps[:], 0.0)
    else:
        nc.scalar.activation(hp2[:, o:o + n], hps[:], AF.Relu)
    nc.tensor.matmul(sc_ps[i][:], w2_sb[:, (c * BL + b) * BL:(c * BL + b + 1) * BL],
                     hp2[:, o:o + n], start=(tid == 0), stop=last)


def _np_dt():
    return ml_dtypes.bfloat16 if DT_MODE == "bf16" else np.float32


def _bir_dt():
    return mybir.dt.bfloat16 if DT_MODE == "bf16" else mybir.dt.float32r


def _patch_act_tables():
    from concourse import hw_specs
    orig = hw_specs.get_activation_tables

    def patched(arch):
        return orig(arch)

    bacc.get_activation_tables = patched


def _build_program():
    _patch_act_tables()
    DT = _bir_dt()
    nc = bacc.Bacc()

    def par(name, shape, dt=None):
        return nc.declare_dram_parameter(name, list(shape), dt or DT, isOutput=False)

    # per-core activations, [128, chunks*tok] partition-major folded layouts
    xt = par("xt", [128, KT * TT])
    xi = par("xi", [128, KI * TI])
    # weights, [128, chunks*512]
    wt = par("wt", [128, KT * D])
    wi = par("wi", [128, KI * D])
    w1t = par("w1t", [128, KD * D])
    w1i = par("w1i", [128, KD * D])
    # one-hot w2 columns: [128, (c, b, 4)]
    w2oh = par("w2oh", [128, KD * BL * BL])
    # per-partition bias/scale columns (f32)
    btx = par("btx", [128, KD], F32)
    bim = par("bim", [128, KD], F32)
    gim = par("gim", [128, KD], F32)
    beE = par("beE", [128, KD], F32)
    b2c = par("b2c", [BL, 1], F32)
    biasrow_p = par("biasrow", [1, BL * D])
    indER_p = par("indER", [128, ER])
    epsc = par("epsc", [1, 1], F32)
    halfc = par("halfc", [BL, 1], F32)
    ones_c = par("ones_c", [128, 1])
    ones_r = par("ones_r", [1, 128])
    ident = par("ident", [128, 128])

    warm_out = nc.declare_dram_parameter("warm_out", [1, D], F32, isOutput=True)
    scores_out = nc.declare_dram_parameter("scores_out", [BL, E, R], F32, isOutput=True)
    agg_out = nc.declare_dram_parameter("agg_out", [BL, D], F32, isOutput=True)

    with ExitStack() as ctx:
        tc = tile.TileContext(nc)
        ctx.enter_context(tc)
        sb = ctx.enter_context(tc.tile_pool(name="sb", bufs=1))
        pp = tc.alloc_tile_pool(name="pp", bufs=2, space="PSUM")
        pstat = tc.alloc_tile_pool(name="pstat", bufs=1, space="PSUM")

        def load(name, p, shape, dt=None):
            t = sb.tile(list(shape), dt or DT, name=name)
            nc.sync.dma_start(t[:], p[:])
            return t

        onesc_sb = load("onesc_sb", ones_c, [128, 1])
        xi_sb = load("xi_sb", xi, [128, KI * TI])
        wi_sb = load("wi_sb", wi, [128, KI * D])
        xt_sb = load("xt_sb", xt, [128, KT * TT])
        wt_sb = load("wt_sb", wt, [128, KT * D])
        bim_sb = load("bim_sb", bim, [128, KD], F32)
        btx_sb = load("btx_sb", btx, [128, KD], F32)
        onesr_sb = load("onesr_sb", ones_r, [1, 128])
        eps_sb = load("eps_sb", epsc, [1, 1], F32)
        w1i_sb = load("w1i_sb", w1i, [128, KD * D])
        w1t_sb = load("w1t_sb", w1t, [128, KD * D])
        indER_sb = load("indER_sb", indER_p, [128, ER])
        w2_sb = load("w2_sb", w2oh, [128, KD * BL * BL])
        gim_sb = load("gim_sb", gim, [128, KD], F32)
        beE_sb = load("beE_sb", beE, [128, KD], F32)
        b2_sb = load("b2_sb", b2c, [BL, 1], F32)
        half_sb = load("half_sb", halfc, [BL, 1], F32)
        id_sb = load("id_sb", ident, [128, 128])

        # HAM warmup while input DMAs run: dense dummy matmuls
        wps = pp.tile([1, D], F32, name="wps", tag="mmi", bufs=1)
        for i in range(12):
            nc.tensor.matmul(wps[:], onesc_sb[:], xi_sb[0:128, 0:D],
                             start=(i == 0), stop=(i == 11))
        warm_sb = sb.tile([1, D], F32)
        nc.scalar.copy(warm_sb[:], wps[:])
        nc.sync.dma_start(warm_out[:], warm_sb[:])

        def proj_phase(x_sb, w_sb, bias_col, kch, tok, tag):
            """x (transposed, chunked) @ W -> relu -> z; stat sums on PE."""
            z = sb.tile([128, KD * tok], DT, name=f"z_{tag}")
            for m in range(KD):
                ps = pp.tile([128, tok], F32, name=f"ps_{tag}", tag="mm")
                for k in range(kch):
                    nc.tensor.matmul(
                        ps[:],
                        w_sb[:, k * D + m * 128:k * D + (m + 1) * 128],
                        x_sb[:, k * tok:(k + 1) * tok],
                        start=(k == 0), stop=(k == kch - 1),
                    )
                nc.scalar.activation(
                    z[:, m * tok:(m + 1) * tok], ps[:], AF.Relu,
                    bias=bias_col[:, m:m + 1], scale=1.0,
                )
            zsq = sb.tile([128, KD * tok], DT, name=f"zsq_{tag}")
            for m in range(KD):
                sl = slice(m * tok, (m + 1) * tok)
                nc.vector.tensor_tensor(zsq[:, sl], z[:, sl], z[:, sl], op=OP.mult)
            s12 = pstat.tile([1, 2 * tok], F32, name=f"s12_{tag}", tag=f"s12_{tag}")
            s1 = s12[:, 0:tok]
            s2 = s12[:, tok:2 * tok]
            for m in range(KD):
                nc.tensor.matmul(s1, onesc_sb[:], z[:, m * tok:(m + 1) * tok],
                                 start=(m == 0), stop=(m == KD - 1))
            for m in range(KD):
                nc.tensor.matmul(s2, onesc_sb[:], zsq[:, m * tok:(m + 1) * tok],
                                 start=(m == 0), stop=(m == KD - 1))
            return z, s1, s2

        def ln_var(s1, s2, tok, tag):
            msq = sb.tile([1, tok], F32, name=f"msq_{tag}")
            nc.scalar.activation(msq[:], s1, AF.Square, bias=0.0, scale=1.0 / D)
            var = sb.tile([1, tok], F32, name=f"var_{tag}")
            nc.vector.scalar_tensor_tensor(var[:], s2, 1.0 / D, msq[:],
                                           op0=OP.mult, op1=OP.subtract)
            return var

        def ln_norm(z, s1, a_row, tok, tag):
            c_row = sb.tile([1, tok], DT, name=f"c_{tag}")
            nc.vector.scalar_tensor_tensor(c_row[:], s1, -1.0 / D, a_row[:],
                                           op0=OP.mult, op1=OP.mult)
            psA = pstat.tile([128, tok], F32, name=f"psA_{tag}", tag="psA")
            psC = pstat.tile([128, tok], F32, name=f"psC_{tag}", tag="psC")
            nc.tensor.matmul(psA[:], onesr_sb[:], a_row[:], start=True, stop=True)
            nc.tensor.matmul(psC[:], onesr_sb[:], c_row[:], start=True, stop=True)
            zn = sb.tile([128, KD * tok], DT, name=f"zn_{tag}")
            for m in range(KD):
                tmp = sb.tile([128, tok], F32, name=f"tmp_{tag}", tag=f"tmp_{tag}", bufs=2)
                nc.vector.tensor_tensor(tmp[:], z[:, m * tok:(m + 1) * tok], psA[:],
                                        op=OP.mult)
                nc.vector.tensor_tensor(zn[:, m * tok:(m + 1) * tok], tmp[:], psC[:],
                                        op=OP.add)
            return zn

        def ln_rstd(var, tok, tag):
            lv = sb.tile([1, tok], F32, name=f"lv_{tag}")
            nc.scalar.activation(lv[:], var[:], AF.Ln, bias=eps_sb[0:1, 0:1], scale=1.0)
            a_row = sb.tile([1, tok], DT, name=f"a_{tag}")
            nc.scalar.activation(a_row[:], lv[:], AF.Exp, bias=0.0, scale=-0.5)
            return a_row

        z_i, s1_i, s2_i = proj_phase(xi_sb, wi_sb, bim_sb, KI, TI, "i")
        z_t, s1_t, s2_t = proj_phase(xt_sb, wt_sb, btx_sb, KT, TT, "t")
        var_i = ln_var(s1_i, s2_i, TI, "i")
        a_i = ln_rstd(var_i, TI, "i")
        zn_i = ln_norm(z_i, s1_i, a_i, TI, "i")
        var_t = ln_var(s1_t, s2_t, TT, "t")
        a_t = ln_rstd(var_t, TT, "t")
        zn_t = ln_norm(z_t, s1_t, a_t, TT, "t")

        # h_t / h_i in standard [token, dhat] layout, packed into one tile:
        # pack[:, b*D + dh]: rows 0-63 = h_t[b, e, dh], rows 64-99 = h_i[b, r, dh],
        # row 100 = bias_ht + bias_hi (added via the indicator's ones row),
        # rows 101+ zeroed (indicator rows there are zero anyway).
        pack = sb.tile([128, BL * D], DT)
        nc.vector.memset(pack[96:128, :], 0.0)
        nc.sync.dma_start(pack[100:101, :], biasrow_p[:])
        for b in range(BL):
            ps = pp.tile([36, D], F32, name="ps_hi", tag="mmi", bufs=1)
            for k in range(KD):
                nc.tensor.matmul(ps[:], zn_i[:, k * TI + b * R:k * TI + (b + 1) * R],
                                 w1i_sb[:, k * D:(k + 1) * D],
                                 start=(k == 0), stop=(k == KD - 1))
            nc.vector.tensor_copy(pack[64:100, b * D:(b + 1) * D], ps[:])
        for mt in range(2):  # text token tiles (128 tokens = 2 samples each)
            ps = pp.tile([128, D], F32, name="ps_ht", tag="mm")
            for k in range(KD):
                nc.tensor.matmul(ps[:], zn_t[:, k * TT + mt * 128:k * TT + (mt + 1) * 128],
                                 w1t_sb[:, k * D:(k + 1) * D],
                                 start=(k == 0), stop=(k == KD - 1))
            for j in range(2):
                b = 2 * mt + j
                nc.vector.tensor_copy(pack[0:64, b * D:(b + 1) * D],
                                      ps[j * 64:(j + 1) * 64, :])

        pstat.release()
        pp.release()
        psc = ctx.enter_context(tc.tile_pool(name="psc", bufs=1, space="PSUM"))
        php = ctx.enter_context(tc.tile_pool(name="php", bufs=3, space="PSUM"))
        # pairwise: H = h_t[e] + h_i[r] + bias via one indicator matmul per
        # (b, c, ntile); relu-evac from PSUM; then w2 one-hot dot into sc_ps.
        NT = [(0, 512), (512, 512), (1024, 512), (1536, 512), (2048, 256)]
        sc_ps = [psc.tile([BL, n], F32, name=f"sc{i}", tag=f"sc{i}")
                 for i, (o, n) in enumerate(NT)]
        hp_pool = ctx.enter_context(tc.tile_pool(name="hp", bufs=3))
        # software-pipelined at H-slice granularity so the dot matmul for
        # slice k never blocks the expansion matmul for slice k+1 in PE's FIFO
        slices = []
        for b in range(BL):
            for c in range(KD):
                tid = b * KD + c
                hp2 = hp_pool.tile([128, ER], DT, name="hp2", tag="hp2")
                for i, (o, n) in enumerate(NT):
                    slices.append((tid, b, c, hp2, i, o, n))
        pend = []
        for si, (tid, b, c, hp2, i, o, n) in enumerate(slices):
            hps = php.tile([128, n], F32, name="hps", tag="H")
            nc.tensor.matmul(hps[:], pack[:, b * D + c * 128:b * D + (c + 1) * 128],
                             indER_sb[:, o:o + n], start=True, stop=True)
            pend.append((tid, b, c, hp2, i, o, n, hps))
            if len(pend) >= 3:
                _emit_slice(nc, pend.pop(0), sc_ps, w2_sb, si)
        for k, item in enumerate(pend):
            _emit_slice(nc, item, sc_ps, w2_sb, k)

        tanh_sb = sb.tile([BL, ER], F32)
        for i, (o, n) in enumerate(NT):
            nc.scalar.activation(tanh_sb[:, o:o + n], sc_ps[i][:], AF.Tanh,
                                 bias=b2_sb[:, 0:1], scale=0.5)
        scores_sb = sb.tile([BL, ER], F32)
        nc.vector.tensor_scalar(scores_sb[:], tanh_sb[:], 0.5, op0=OP.mult,
                                scalar2=0.5, op1=OP.add)
        nc.sync.dma_start(scores_out[:], scores_sb[:].rearrange("b (e r) -> b e r", e=E))

        # softmax weights folded into u; exp(0.5*tanh + 0.5)
        exp_sb = sb.tile([BL, ER], DT)
        nc.scalar.activation(exp_sb[:], tanh_sb[:], AF.Exp,
                             bias=half_sb[:, 0:1], scale=0.5)
        u = sb.tile([BL, R], F32)
        nc.vector.tensor_reduce(u[:], exp_sb[:].rearrange("b (e r) -> b r e", e=E),
                                axis=mybir.AxisListType.X, op=OP.add)
        den = sb.tile([BL, 1], F32)
        nc.vector.tensor_reduce(den[:], u[:], axis=mybir.AxisListType.X, op=OP.add)
        rden = sb.tile([BL, 1], F32)
        nc.vector.reciprocal(rden[:], den[:])
        u_f = sb.tile([BL, R], DT)
        nc.vector.tensor_scalar(u_f[:], u[:], rden[:, 0:1], op0=OP.mult,
                                scalar2=1.0 / E, op1=OP.mult)
        u_row = sb.tile([1, TI], DT)
        nc.sync.dma_start(u_row[0:1, :].rearrange("q (b r) -> q b r", b=BL), u_f[:])

        psU = php.tile([128, TI], F32, tag="H")
        nc.tensor.matmul(psU[:], onesr_sb[:], u_row[:], start=True, stop=True)
        aggT = sb.tile([128, KD * BL], F32)
        for c in range(KD):
            tmp = sb.tile([128, TI], F32, name="agg_tmp", tag="agg_tmp", bufs=2)
            nc.vector.tensor_tensor(tmp[:], zn_i[:, c * TI:(c + 1) * TI], psU[:],
                                    op=OP.mult)
            nc.vector.tensor_reduce(
                aggT[:, c * BL:(c + 1) * BL],
                tmp[:].rearrange("p (b r) -> p b r", b=BL),
                axis=mybir.AxisListType.X, op=OP.add)
        aggF = sb.tile([128, KD * BL], DT)
        for c in range(KD):
            nc.vector.scalar_tensor_tensor(
                aggF[:, c * BL:(c + 1) * BL], aggT[:, c * BL:(c + 1) * BL],
                gim_sb[:, c:c + 1],
                beE_sb[:, c:c + 1].broadcast_to([128, BL]),
                op0=OP.mult, op1=OP.add)
        psT = php.tile([KD * BL, 128], DT, tag="H")
        nc.tensor.transpose(psT[:], aggF[:], id_sb[:])
        agg_sb = sb.tile([KD * BL, 128], F32)
        nc.scalar.copy(agg_sb[:], psT[:])
        # row (c*BL + b) -> agg_out[b, c*128 : (c+1)*128]
        for c in range(KD):
            nc.sync.dma_start(agg_out[:, c * 128:(c + 1) * 128],
                              agg_sb[c * BL:(c + 1) * BL, :])

    nc.compile()
    return nc


def _fold(a, nchunk):
    """[nchunk*128, cols] -> [128, nchunk*cols] partition-major layout."""
    n, cols = a.shape
    assert n == nchunk * 128
    return np.ascontiguousarray(
        a.reshape(nchunk, 128, cols).transpose(1, 0, 2).reshape(128, nchunk * cols))


def _prep_host(inputs):
    npdt = _np_dt()
    f32 = np.float32

    text = np.asarray(inputs["text_feats"], f32)
    image = np.asarray(inputs["image_feats"], f32)
    W_text = np.asarray(inputs["W_text"], f32)
    b_text = np.asarray(inputs["b_text"], f32)
    g_text = np.asarray(inputs["g_text"], f32)
    beta_text = np.asarray(inputs["beta_text"], f32)
    W_img = np.asarray(inputs["W_img"], f32)
    b_img = np.asarray(inputs["b_img"], f32)
    g_img = np.asarray(inputs["g_img"], f32)
    beta_img = np.asarray(inputs["beta_img"], f32)
    W1 = np.asarray(inputs["W1"], f32)
    b1 = np.asarray(inputs["b1"], f32)
    W2 = np.asarray(inputs["W2"], f32)
    b2 = np.asarray(inputs["b2"], f32)

    W1t = W1[:D] * g_text[:, None]
    W1i = W1[D:] * g_img[:, None]
    bht = beta_text @ W1[:D]
    bhi = beta_img @ W1[D:] + b1

    shared = {
        "wt": _fold(W_text, KT).astype(npdt),
        "wi": _fold(W_img, KI).astype(npdt),
        "w1t": _fold(W1t, KD).astype(npdt),
        "w1i": _fold(W1i, KD).astype(npdt),
        "btx": np.ascontiguousarray(b_text.reshape(KD, 128).T),
        "bim": np.ascontiguousarray(b_img.reshape(KD, 128).T),
        "biasrow": np.tile((bht + bhi), BL).reshape(1, BL * D).astype(npdt),
        "gim": np.ascontiguousarray(g_img.reshape(KD, 128).T),
        "beE": np.ascontiguousarray((beta_img / E).reshape(KD, 128).T),
        "b2c": np.full((BL, 1), b2[0] * 0.5, f32),
        "halfc": np.full((BL, 1), 0.5, f32),
        "epsc": np.full((1, 1), LN_EPS, f32),
        "ones_c": np.ones((128, 1), f32).astype(npdt),
        "ones_r": np.ones((1, 128), f32).astype(npdt),
        "ident": np.eye(128, dtype=f32).astype(npdt),
    }
    # one-hot w2: [128, (c, b, col)] where col b holds w2 chunk c
    w2f = W2[:, 0].reshape(KD, 128)
    w2oh = np.zeros((128, KD, BL, BL), f32)
    for c in range(KD):
        for b in range(BL):
            w2oh[:, c, b, b] = w2f[c]
    shared["w2oh"] = w2oh.reshape(128, KD * BL * BL).astype(npdt)
    ind = np.zeros((128, E, R), f32)
    for e in range(E):
        ind[e, e, :] = 1.0
    for r in range(R):
        ind[64 + r, :, r] = 1.0
    ind[100, :, :] = 1.0
    shared["indER"] = ind.reshape(128, ER).astype(npdt)

    in_maps = []
    for core in range(NCORES):
        sl = slice(core * BL, (core + 1) * BL)
        xt = _fold(np.ascontiguousarray(text[sl].reshape(TT, Dt).T), KT)
        xi = _fold(np.ascontiguousarray(image[sl].reshape(TI, Di).T), KI)
        m = dict(shared)
        m["xt"] = xt.astype(npdt)
        m["xi"] = xi.astype(npdt)
        in_maps.append(m)
    return in_maps


def _install_ntff_hook():
    """The slim container lacks antenv.axon_hooks; recreate it so
    run_bass_kernel_spmd(trace=True) can capture NTFF profiles."""
    import types, ctypes, contextlib

    try:
        from antenv.axon_hooks import get_axon_ntff_profile_hook  # noqa: F401
        return
    except ImportError:
        pass
    so_path = "/opt/axon/libaxon_pjrt.so"
    try:
        lib = ctypes.CDLL(so_path)
    except OSError:
        return
    if not hasattr(lib, "axon_start_nrt_profile"):
        return
    lib.axon_start_nrt_profile.argtypes = [ctypes.POINTER(ctypes.c_int64), ctypes.c_size_t]
    lib.axon_start_nrt_profile.restype = ctypes.c_int64
    lib.axon_stop_nrt_profile.argtypes = [ctypes.c_char_p]
    lib.axon_stop_nrt_profile.restype = ctypes.c_int64

    @contextlib.contextmanager
    def _hook(output_dir, device_ids):
        import jax
        jax.devices()
        if device_ids:
            ids = (ctypes.c_int64 * len(device_ids))(*device_ids)
            rc = lib.axon_start_nrt_profile(ids, len(device_ids))
        else:
            rc = lib.axon_start_nrt_profile(None, 0)
        if rc != 0:
            raise RuntimeError(f"axon_start_nrt_profile rc={rc}")
        try:
            yield
        finally:
            n = lib.axon_stop_nrt_profile(str(output_dir).encode())
            print(f"ntff profile: {n} file(s) -> {output_dir}", file=sys.stderr)

    mod = types.ModuleType("antenv.axon_hooks")
    mod.get_axon_ntff_profile_hook = lambda: _hook
    mod.set_axon_ntff_profile_hook = lambda h: None
    sys.modules["antenv.axon_hooks"] = mod
    import antenv
    antenv.axon_hooks = mod


def _get_program():
    key = DT_MODE
    if key not in _CACHE:
        _CACHE[key] = _build_program()
    return _CACHE[key]


def kernel(trace=False, **inputs):
    if trace:
        _install_ntff_hook()
    nc = _get_program()
    in_maps = _prep_host(inputs)
    res = run_bass_kernel_spmd(nc, in_maps, list(range(NCORES)), trace=trace)
    scores = np.concatenate([res.results[c]["scores_out"] for c in range(NCORES)], axis=0)
    agg = np.concatenate([res.results[c]["agg_out"] for c in range(NCORES)], axis=0)
    out = (scores.astype(np.float32), agg.astype(np.float32))
    if trace:
        return out, res
    return out
